# revision 11
# baseline (speedup 1.0000x reference)
"""Trainium2 Bass kernel for nn_Graph_CNN_Feat_Mesh (Chebyshev GNN decoder).

Strategy (per-core, data-parallel over batch B=256 -> 32/core):
  - All spmms are dense matmuls on the tensor engine (PE) in bf16:
      y = A + L @ (B + L @ (2C)),  A/B/C = feature-space linears of the input.
    L is densified on host; for up4-preceded layers the replication is folded
    into LU = L @ U (contracting the small pre-upsample vertex space), and the
    A/B-linear inputs are read through stride-0 broadcast APs (no copies).
  - B and A linear terms accumulate directly into the spmm PSUM.
  - Activations live in packed F-layout [(j,Fin) partitions, (b//G)*Vsp + v]
    between layers; the per-layer linear emits V-layout directly; one PE
    transpose per layer returns to F-layout, with bn_stats chunks fused in.
  - BatchNorm (training mode, global batch stats) is exact: per-core partial
    (sum, sumsq) go through an AllGather (cheaper than AllReduce) and are
    tree-reduced locally; scale/shift+relu is applied per vertex-tile chunk so
    the next layer's PE work starts immediately.
  - FC head (2048->512->5120) runs fully in bf16 with fp32 PSUM accumulation;
    weights are pre-tiled on host into single-DMA layouts.
"""

import numpy as np

B = 256
NCORES = 8
BL = B // NCORES  # 32
EPS = 1e-5

_CACHE = {}


def _split_W(W):
    W = np.asarray(W, np.float32)
    return W[:, 0::3], W[:, 1::3], W[:, 2::3]


def _dense_L(rows, cols, vals, V):
    L = np.zeros((V, V), np.float32)
    np.add.at(L, (np.asarray(rows), np.asarray(cols)), np.asarray(vals, np.float32))
    return L


def _tile_k(a, tk=128):
    """[K, N] -> [128, (K//128)*N] (k-tile-major columns)."""
    K, N = a.shape
    if K % tk:
        a = np.concatenate([a, np.zeros((tk - K % tk, N), a.dtype)], 0)
    nk = a.shape[0] // tk
    return np.ascontiguousarray(
        a.reshape(nk, tk, N).transpose(1, 0, 2).reshape(tk, nk * N))


class _LCfg:
    def __init__(self, name, Vsp, V, Fin, Fout, up4, bn):
        self.name = name
        self.Vsp = Vsp      # source vertex space of C-linear (pre-up4)
        self.V = V          # output vertex count
        self.Fin = Fin
        self.Fout = Fout
        self.G = 128 // Fin          # batches packed on partitions at input
        self.nG = BL // self.G
        self.GF = self.G * Fout      # N of one B/C/A-linear matmul
        self.Gp = 128 // Fout if Fout in (32, 64) else None
        self.nGp = BL // self.Gp if self.Gp else None
        self.up4 = up4
        self.bn = bn
        self.nVt = (V + 127) // 128
        self.nVsp = (Vsp + 127) // 128
        self.BF = BL * Fout          # free width of V-layout per vtile

    def vts(self, t):
        return min(128, self.V - t * 128)

    def sps(self, s):
        return min(128, self.Vsp - s * 128)


CFGS = [
    _LCfg("c0", 80, 320, 64, 64, True, True),
    _LCfg("c1", 320, 320, 64, 32, False, True),
    _LCfg("c2", 320, 1280, 32, 32, True, True),
    _LCfg("c3", 1280, 1280, 32, 3, False, False),
]


def _wbd(W, G, Fin, Fout, which):
    """Block-diagonal rhs weight [128, G*Fout] for the fused linear.
    which: 'A' -> W0 - W2, 'B' -> W1, 'C' -> 2*W2.  col = j*Fout + c."""
    W0, W1, W2 = _split_W(W)
    M = {"A": W0 - W2, "B": W1, "C": 2.0 * W2}[which]  # [Fout, Fin]
    out = np.zeros((128, G * Fout), np.float32)
    for j in range(G):
        out[j * Fin:(j + 1) * Fin, j * Fout:(j + 1) * Fout] = M.T
    return out


def _build_host(inputs):
    import ml_dtypes
    bf = ml_dtypes.bfloat16
    f32 = np.float32
    d = {}
    xT = np.ascontiguousarray(np.asarray(inputs["x"], f32).T)  # [2048, 256]
    d["xT"] = xT  # sliced + tiled per-core in kernel()
    d["fc1wt"] = _tile_k(np.asarray(inputs["fc1_w"], f32).T).astype(bf)
    d["fc1b"] = np.ascontiguousarray(
        np.asarray(inputs["fc1_b"], f32).reshape(4, 128).T)  # [128,4]
    # fc2: [512, 5120] -> per-mc [512, 1280] k-tiled, mc-major concat
    fc2 = np.asarray(inputs["fc2_w"], f32).T
    d["fc2wt"] = np.concatenate(
        [_tile_k(fc2[:, mc * 1280:(mc + 1) * 1280]) for mc in range(4)],
        axis=1).astype(bf)  # [128, 4*4*1280]

    L1 = _dense_L(inputs["L1_rows"], inputs["L1_cols"], inputs["L1_vals"], 320)
    L2 = _dense_L(inputs["L2_rows"], inputs["L2_cols"], inputs["L2_vals"], 1280)
    U1 = np.repeat(np.eye(80, dtype=f32), 4, axis=0)    # [320, 80]
    U2 = np.repeat(np.eye(320, dtype=f32), 4, axis=0)   # [1280, 320]
    lu0 = (L1 @ U1).T  # [80, 320]
    d["LU0"] = np.concatenate(
        [lu0, np.zeros((48, 320), f32)], 0).astype(bf)   # [128, 320]
    d["LT1t"] = _tile_k(L1.T).astype(bf)                 # [128, 3*320]
    d["LU2t"] = _tile_k((L2 @ U2).T).astype(bf)          # [128, 3*1280]
    d["LT2t"] = _tile_k(L2.T).astype(bf)                 # [128, 10*1280]

    Wn = {"c0": "cl0_w", "c1": "cl1_w", "c2": "cl2_w", "c3": "cl3_w"}
    wall = []
    for cfg in CFGS:
        W = np.asarray(inputs[Wn[cfg.name]], f32)
        for which in "ABC":
            wall.append(_wbd(W, cfg.G, cfg.Fin, cfg.Fout, which))
    d["Wall"] = np.concatenate(wall, axis=1).astype(bf)  # [128, 3*(128+64+128+12)]
    d["b3"] = np.asarray(inputs["cl3_b"], f32).copy()

    for i, (g, b) in enumerate([("bn0_g", "bn0_b"), ("bn1_g", "bn1_b"),
                                ("bn2_g", "bn2_b")]):
        gb = np.concatenate([np.asarray(inputs[g], f32),
                             np.asarray(inputs[b], f32)])
        d[f"gb{i}"] = np.ascontiguousarray(gb[None, :])  # [1, 2F]

    for F, nm in [(64, "sel64"), (32, "sel32")]:
        Gp = 128 // F
        sel = np.zeros((128, F), f32)
        for j in range(Gp):
            sel[j * F:(j + 1) * F] += np.eye(F, dtype=f32)
        d[nm] = sel
    return d


def _build_nc(b3_imm):
    import sys
    for p in ("/opt/trn_rl_repo", "/opt/trn_rl_repo/concourse"):
        if p not in sys.path:
            sys.path.insert(0, p)
    import concourse.bass as bass  # noqa
    import concourse.mybir as mybir
    import concourse.tile as tile
    from concourse import bacc
    from concourse.masks import make_identity

    f32 = mybir.dt.float32
    bf16 = mybir.dt.bfloat16
    AF = mybir.ActivationFunctionType
    ALU = mybir.AluOpType

    nc = bacc.Bacc(None, target_bir_lowering=False)

    xT = nc.dram_tensor("xTt", [128, 16 * BL], bf16, kind="ExternalInput")
    fc1wt = nc.dram_tensor("fc1wt", [128, 16 * 512], bf16, kind="ExternalInput")
    fc1b = nc.dram_tensor("fc1b", [128, 4], f32, kind="ExternalInput")
    fc2wt = nc.dram_tensor("fc2wt", [128, 16 * 1280], bf16, kind="ExternalInput")
    LU0 = nc.dram_tensor("LU0", [128, 320], bf16, kind="ExternalInput")
    LT1t = nc.dram_tensor("LT1t", [128, 3 * 320], bf16, kind="ExternalInput")
    LU2t = nc.dram_tensor("LU2t", [128, 3 * 1280], bf16, kind="ExternalInput")
    LT2t = nc.dram_tensor("LT2t", [128, 10 * 1280], bf16, kind="ExternalInput")
    WCOLS = sum(cfg.GF for cfg in CFGS) * 3
    Wallt = nc.dram_tensor("Wall", [128, WCOLS], bf16, kind="ExternalInput")
    gbs = [nc.dram_tensor(f"gb{i}", [1, 2 * F], f32, kind="ExternalInput")
           for i, F in enumerate([64, 32, 32])]
    sel64 = nc.dram_tensor("sel64", [128, 64], f32, kind="ExternalInput")
    sel32 = nc.dram_tensor("sel32", [128, 32], f32, kind="ExternalInput")
    ydram = nc.dram_tensor("y", [BL, 1280 * 3], f32, kind="ExternalOutput")

    with tile.TileContext(nc) as tc:
        with (
            tc.tile_pool(name="const", bufs=1) as constp,
            tc.tile_pool(name="wpool", bufs=1) as wpool,
            tc.tile_pool(name="headp", bufs=1) as headp,
            tc.tile_pool(name="poolA", bufs=2) as poolA,
            tc.tile_pool(name="poolB", bufs=1) as poolB,
            tc.tile_pool(name="poolC", bufs=1) as poolC,
            tc.tile_pool(name="misc", bufs=1) as miscp,
            tc.tile_pool(name="outp", bufs=3) as outp,
            tc.tile_pool(name="pslin", bufs=2, space="PSUM") as pslin,
            tc.tile_pool(name="psbig", bufs=2, space="PSUM") as psbig,
            tc.tile_pool(name="pstr", bufs=2, space="PSUM") as pstr,
            tc.tile_pool(name="dram", bufs=1, space="DRAM") as dramp,
        ):
            # ---- critical-path loads first: FC head operands ----
            xT_sb = headp.tile([128, 16 * BL], bf16, tag="xT")
            nc.sync.dma_start(xT_sb[:], xT[:])
            fc1b_sb = constp.tile([128, 4], f32, tag="fc1b")
            nc.sync.dma_start(fc1b_sb[:], fc1b[:])
            fc1w_sb = headp.tile([128, 16 * 512], bf16, tag="fc1w")
            nc.sync.dma_start(fc1w_sb[:], fc1wt[:])
            fc2w_sb = headp.tile([128, 16 * 1280], bf16, tag="fc2w")
            nc.sync.dma_start(fc2w_sb[:, :8 * 1280], fc2wt[:, :8 * 1280])

            # ---- secondary loads (stream while head computes) ----
            ident_b = constp.tile([128, 128], bf16, tag="identb")
            make_identity(nc, ident_b[:])
            ident_f = constp.tile([128, 128], f32, tag="identf")
            make_identity(nc, ident_f[:])
            sel_sb = {64: constp.tile([128, 64], f32, tag="sel64", name="sel64sb"),
                      32: constp.tile([128, 32], f32, tag="sel32", name="sel32sb")}
            nc.scalar.dma_start(sel_sb[64][:], sel64[:])
            nc.scalar.dma_start(sel_sb[32][:], sel32[:])
            gb_sb = []
            for i, F in enumerate([64, 32, 32]):
                t = constp.tile([1, 2 * F], f32, tag=f"gb{i}")
                nc.scalar.dma_start(t[:], gbs[i][:])
                gb_sb.append(t)
            eps_t = constp.tile([1, 1], f32, tag="eps")
            nc.gpsimd.memset(eps_t[:], EPS)

            W_sb = {}
            wall_sb = wpool.tile([128, WCOLS], bf16, tag="Wall")
            nc.scalar.dma_start(wall_sb[:], Wallt[:])
            woff = 0
            for cfg in CFGS:
                for w in "ABC":
                    W_sb[f"{w}{cfg.name}"] = wall_sb[:, woff:woff + cfg.GF]
                    woff += cfg.GF

            LUT, LT = {}, {}
            t = wpool.tile([128, 320], bf16, tag="LU0")
            nc.scalar.dma_start(t[:], LU0[:])
            LUT["c0"] = t
            t = wpool.tile([128, 3 * 320], bf16, tag="LT1")
            nc.scalar.dma_start(t[:], LT1t[:])
            LT["c0"] = LT["c1"] = LUT["c1"] = t
            nc.sync.dma_start(fc2w_sb[:, 8 * 1280:], fc2wt[:, 8 * 1280:])
            t = wpool.tile([128, 3 * 1280], bf16, tag="LU2")
            nc.scalar.dma_start(t[:], LU2t[:])
            LUT["c2"] = t
            t = wpool.tile([128, 10 * 1280], bf16, tag="LT2")
            nc.scalar.dma_start(t[:], LT2t[:])
            LT["c2"] = LT["c3"] = LUT["c3"] = t

            # ================= FC head (bf16) =================
            h1T = headp.tile([128, 4 * BL], bf16, tag="h1T")
            ps1 = pslin.tile([128, 512], f32, tag="lin")
            for mt in range(4):
                for kt in range(16):
                    nc.tensor.matmul(
                        ps1[:, mt * BL:(mt + 1) * BL],
                        fc1w_sb[:, kt * 512 + mt * 128: kt * 512 + (mt + 1) * 128],
                        xT_sb[:, kt * BL:(kt + 1) * BL],
                        start=(kt == 0), stop=(kt == 15))
            for mt in range(4):
                nc.scalar.activation(
                    h1T[:, mt * BL:(mt + 1) * BL], ps1[:, mt * BL:(mt + 1) * BL],
                    AF.Relu, bias=fc1b_sb[:, mt:mt + 1])

            # fc2 in 4 column-chunks of 1280 (10 m-tiles each).
            # psum partition = (v0%2)*64+f, col = mi*BL+b ; channels c = v0*64+f.
            # dest: XF0[(b%2)*64+f, (b//2)*80 + v0],  v0 = 2*(mc*10+mi)+p0
            XF0 = poolC.tile([128, 16 * 80], bf16, tag="XF0")
            for mc in range(4):
                ps2 = pslin.tile([128, 512], f32, tag="lin")
                for mi in range(10):
                    for kt in range(4):
                        nc.tensor.matmul(
                            ps2[:, mi * BL:(mi + 1) * BL],
                            fc2w_sb[:, (mc * 4 + kt) * 1280 + mi * 128:
                                    (mc * 4 + kt) * 1280 + (mi + 1) * 128],
                            h1T[:, kt * BL:(kt + 1) * BL],
                            start=(kt == 0), stop=(kt == 3))
                src4 = ps2[:, :10 * BL].rearrange("p (i g j) -> p i g j", g=16, j=2)
                dst4 = XF0[:].rearrange("p (g u q) -> p g u q", u=40, q=2)
                for p0 in range(2):
                    for j in range(2):
                        nc.scalar.activation(
                            dst4[j * 64:(j + 1) * 64, :,
                                 mc * 10:(mc + 1) * 10, p0]
                            .rearrange("p g i -> p i g"),
                            src4[p0 * 64:(p0 + 1) * 64, :, :, j],
                            AF.Copy)

            # ================= cheby layers =================
            XF_cur = XF0
            ar_idx = 0

            for li, cfg in enumerate(CFGS):
                V, Vsp, F = cfg.V, cfg.Vsp, cfg.Fout
                BF = cfg.BF
                last = cfg.name == "c3"

                def rep_in(g, t, vsz, _X=XF_cur, _cfg=cfg):
                    """A/B-linear input columns [g*V + t*128, +vsz] of the
                    (virtually) up4-replicated XF_cur."""
                    if not _cfg.up4:
                        return _X[:, g * _cfg.V + t * 128:
                                  g * _cfg.V + t * 128 + vsz]
                    u0 = t * 32
                    sl = _X[:, g * _cfg.Vsp + u0: g * _cfg.Vsp + u0 + vsz // 4]
                    return sl.unsqueeze(2).broadcast_to((128, vsz // 4, 4))

                # --- C linear (in Vsp space) ---
                XC = poolC.tile([128, cfg.nVsp * BL * F], bf16, tag="XC")
                gpack = max(1, 512 // cfg.GF)
                for s in range(cfg.nVsp):
                    ssz = cfg.sps(s)
                    for g0 in range(0, cfg.nG, gpack):
                        gn = min(gpack, cfg.nG - g0)
                        pc = pslin.tile([128, 512], f32, tag="lin")
                        for gi in range(gn):
                            g = g0 + gi
                            nc.tensor.matmul(
                                pc[:ssz, gi * cfg.GF:(gi + 1) * cfg.GF],
                                XF_cur[:, g * Vsp + s * 128:
                                       g * Vsp + s * 128 + ssz],
                                W_sb[f"C{cfg.name}"],
                                start=True, stop=True)
                        nc.scalar.activation(
                            XC[:ssz, s * BL * F + g0 * cfg.GF:
                               s * BL * F + (g0 + gn) * cfg.GF],
                            pc[:ssz, :gn * cfg.GF], AF.Copy)

                # --- inner = LU @ (2C) + B ;  y = L @ inner + A ---
                Xin = poolB.tile([128, cfg.nVt * BF], bf16, tag="B")
                ytile = poolC.tile([128, cfg.nVt * BF], bf16, tag="YT")
                for phase in range(2):
                    srcL = LUT[cfg.name] if phase == 0 else LT[cfg.name]
                    nS = cfg.nVsp if phase == 0 else cfg.nVt
                    ssizes = ([cfg.sps(s) for s in range(nS)] if phase == 0
                              else [cfg.vts(s) for s in range(nS)])
                    rhs = XC if phase == 0 else Xin
                    rhs_w = BL * F if phase == 0 else BF
                    Wacc = W_sb[f"B{cfg.name}" if phase == 0 else f"A{cfg.name}"]
                    dst = Xin if phase == 0 else ytile
                    for t in range(cfg.nVt):
                        vsz = cfg.vts(t)
                        for pc0 in range(0, BF, 1024):
                            pw = min(1024, BF - pc0)
                            pi = psbig.tile([128, max(pw, 512)], f32, tag="big")
                            for nk in range(0, pw, 512):
                                n0 = pc0 + nk
                                n1 = min(n0 + 512, pc0 + pw)
                                for s in range(nS):
                                    ssz = ssizes[s]
                                    nc.tensor.matmul(
                                        pi[:vsz, n0 - pc0:n1 - pc0],
                                        srcL[:ssz, s * V + t * 128:
                                             s * V + t * 128 + vsz],
                                        rhs[:ssz, s * rhs_w + n0:
                                            s * rhs_w + n1],
                                        start=(s == 0), stop=False,
                                        skip_group_check=True)
                                for g in range(n0 // cfg.GF,
                                               (n1 + cfg.GF - 1) // cfg.GF):
                                    nc.tensor.matmul(
                                        pi[:vsz, g * cfg.GF - pc0:
                                           (g + 1) * cfg.GF - pc0],
                                        rep_in(g, t, vsz),
                                        Wacc,
                                        start=False, stop=True,
                                        skip_group_check=True)
                            if last and phase == 1:
                                # reorder (b,fo) -> (fo,b) for output staging
                                nc.vector.tensor_copy(
                                    dst[:vsz, t * BF + pc0: t * BF + pc0 + pw]
                                    .rearrange("p (c b) -> p c b", b=BL),
                                    pi[:vsz, :pw]
                                    .rearrange("p (b c) -> p c b", c=3))
                            elif phase == 0:
                                nc.scalar.activation(
                                    dst[:vsz, t * BF + pc0: t * BF + pc0 + pw],
                                    pi[:vsz, :pw], AF.Copy)
                            else:
                                nc.vector.tensor_copy(
                                    dst[:vsz, t * BF + pc0: t * BF + pc0 + pw],
                                    pi[:vsz, :pw])

                if not last:
                    # --- back-transpose to packed F-layout of next level,
                    #     with bn_stats chunks fused in ---
                    Gp, nGp = cfg.Gp, cfg.nGp
                    FD = nGp * V
                    nch = cfg.nVt * nGp
                    bnst = miscp.tile([128, nch * 6], f32, tag="bnst")
                    ch = 0
                    XFn = poolA.tile([128, nGp * V], bf16, tag="A")
                    dstv = XFn[:].rearrange("p (g v) -> p g v", v=V)
                    for t in range(cfg.nVt):
                        vsz = cfg.vts(t)
                        for q0 in range(0, nGp, 4):
                            qn = min(4, nGp - q0)
                            pt = pstr.tile([128, 512], bf16, tag="tr")
                            for qi in range(qn):
                                gp = q0 + qi
                                nc.tensor.transpose(
                                    pt[:, qi * 128: qi * 128 + vsz],
                                    ytile[:vsz, t * BF + gp * 128:
                                          t * BF + (gp + 1) * 128],
                                    ident_b[:vsz, :vsz])
                            dreg = dstv[:, q0:q0 + qn, t * 128:t * 128 + vsz]
                            nc.scalar.activation(
                                dreg,
                                pt[:].rearrange("p (q v) -> p q v", v=128)
                                [:, :qn, :vsz],
                                AF.Copy)
                            nc.vector.bn_stats(
                                bnst[:, ch * 6:(ch + qn) * 6], dreg)
                            ch += qn

                    # --- local (S1, S2) partials -> DRAM ---
                    aggr = miscp.tile([128, 2], f32, tag="aggr")
                    nc.vector.bn_aggr(
                        aggr[:], bnst[:].rearrange("p (c s) -> p c s", s=6))
                    part = miscp.tile([128, 2], f32, tag="part")
                    nc.vector.tensor_tensor(
                        out=part[:, 1:2], in0=aggr[:, 0:1], in1=aggr[:, 0:1],
                        op=ALU.mult)
                    nc.vector.tensor_tensor(
                        out=part[:, 1:2], in0=part[:, 1:2], in1=aggr[:, 1:2],
                        op=ALU.add)
                    nc.vector.tensor_scalar_mul(part[:, 1:2], part[:, 1:2],
                                                float(FD))
                    nc.vector.tensor_scalar_mul(part[:, 0:1], aggr[:, 0:1],
                                                float(FD))
                    pst = pslin.tile([128, 512], f32, tag="lin")
                    nc.tensor.matmul(pst[:1, :F], part[:, 0:1], sel_sb[F][:],
                                     start=True, stop=True)
                    nc.tensor.matmul(pst[:1, F:2 * F], part[:, 1:2],
                                     sel_sb[F][:], start=True, stop=True)
                    stats_l = miscp.tile([1, 2 * F], f32, tag="statl")
                    nc.vector.tensor_copy(stats_l[:], pst[:1, :2 * F])
                    bin_ = dramp.tile([1, 2 * F], f32, tag=f"arin{ar_idx}")
                    bout = dramp.tile([NCORES, 2 * F], f32, tag=f"arout{ar_idx}")
                    nc.gpsimd.dma_start(bin_[:], stats_l[:])
                    nc.gpsimd.collective_compute(
                        "AllGather", ALU.bypass,
                        replica_groups=[list(range(NCORES))],
                        ins=[bin_.opt()], outs=[bout.opt()])
                    gath = miscp.tile([1, NCORES * 2 * F], f32, tag="gath")
                    nc.sync.dma_start(
                        gath[:], bout[:].rearrange("n c -> () (n c)"))
                    # tree-reduce the 8 partials
                    g3 = gath[:].rearrange("p (n c) -> p n c", n=NCORES)
                    red4 = miscp.tile([1, 4 * 2 * F], f32, tag="red4")
                    r4 = red4[:].rearrange("p (n c) -> p n c", n=4)
                    nc.vector.tensor_tensor(out=r4, in0=g3[:, 0:4],
                                            in1=g3[:, 4:8], op=ALU.add)
                    red2 = miscp.tile([1, 2 * 2 * F], f32, tag="red2")
                    r2 = red2[:].rearrange("p (n c) -> p n c", n=2)
                    nc.vector.tensor_tensor(out=r2, in0=r4[:, 0:2],
                                            in1=r4[:, 2:4], op=ALU.add)
                    stats_g = miscp.tile([1, 2 * F], f32, tag="statg")
                    nc.vector.tensor_tensor(out=stats_g[:], in0=red2[:, :2 * F],
                                            in1=red2[:, 2 * F:], op=ALU.add)
                    n_g = float(B * V)
                    # tmp cols [0:F]=mu, [F:2F]=var->rstd ; st [0:F]=s [F:2F]=t
                    st = miscp.tile([1, 2 * F], f32, tag="st")
                    tmp = miscp.tile([1, 2 * F], f32, tag="sttmp")
                    mu2 = miscp.tile([1, F], f32, tag="mu2")
                    nc.vector.tensor_scalar_mul(tmp[:, :2 * F], stats_g[:],
                                                1.0 / n_g)
                    nc.vector.tensor_tensor(out=mu2[:], in0=tmp[:, 0:F],
                                            in1=tmp[:, 0:F], op=ALU.mult)
                    nc.vector.tensor_tensor(out=tmp[:, F:2 * F],
                                            in0=tmp[:, F:2 * F],
                                            in1=mu2[:], op=ALU.subtract)
                    nc.scalar.activation(tmp[:, F:2 * F], tmp[:, F:2 * F],
                                         AF.Sqrt, bias=eps_t[:])
                    nc.vector.reciprocal(tmp[:, F:2 * F], tmp[:, F:2 * F])
                    nc.vector.tensor_tensor(out=st[:, 0:F],
                                            in0=tmp[:, F:2 * F],
                                            in1=gb_sb[li][:, 0:F], op=ALU.mult)
                    nc.vector.tensor_tensor(out=mu2[:], in0=tmp[:, 0:F],
                                            in1=st[:, 0:F], op=ALU.mult)
                    nc.vector.tensor_tensor(out=st[:, F:2 * F],
                                            in0=gb_sb[li][:, F:2 * F],
                                            in1=mu2[:], op=ALU.subtract)
                    pss = pslin.tile([128, 512], f32, tag="lin", name="pss")
                    nc.tensor.transpose(pss[:2 * F, 0:1], st[:],
                                        ident_f[:1, :1])
                    stc = miscp.tile([128, 2], f32, tag=f"stc{ar_idx}")
                    for j in range(Gp):
                        nc.vector.tensor_copy(stc[j * F:(j + 1) * F, 0:1],
                                              pss[:F, 0:1])
                        nc.vector.tensor_copy(stc[j * F:(j + 1) * F, 1:2],
                                              pss[F:2 * F, 0:1])
                    ar_idx += 1
                    # scale+shift+relu per vertex-tile chunk so the next
                    # layer's C-linear (which consumes s-tiles in order)
                    # starts right after chunk 0
                    for s in range(cfg.nVt):
                        vsz = cfg.vts(s)
                        nc.scalar.activation(
                            dstv[:, :, s * 128:s * 128 + vsz],
                            dstv[:, :, s * 128:s * 128 + vsz],
                            AF.Relu, scale=stc[:, 0:1], bias=stc[:, 1:2])
                    XF_cur = XFn
                else:
                    # --- stage output: ytile [v, fo*32+b] -> [b, v*3+fo] ---
                    for t in range(cfg.nVt):
                        pt = pstr.tile([128, 512], bf16, tag="tr")
                        nc.tensor.transpose(
                            pt[:96, :128],
                            ytile[:128, t * BF:(t + 1) * BF],
                            ident_b[:128, :128])
                        och = outp.tile([BL, 384], f32, tag="out")
                        for fo in range(3):
                            nc.vector.tensor_scalar_add(
                                och[:].rearrange("b (v f) -> b v f", f=3)
                                [:, :, fo],
                                pt[fo * 32:(fo + 1) * 32, :128],
                                float(b3_imm[fo]))
                        nc.sync.dma_start(
                            ydram[:, t * 384:(t + 1) * 384], och[:])

    nc.compile()
    return nc


def kernel(**inputs):
    import sys
    for p in ("/opt/trn_rl_repo", "/opt/trn_rl_repo/concourse"):
        if p not in sys.path:
            sys.path.insert(0, p)
    from concourse.bass_utils import run_bass_kernel_spmd

    host = _build_host(inputs)
    b3 = [float(v) for v in host.pop("b3")]

    key = ("nc",) + tuple(b3)
    if key not in _CACHE:
        _CACHE[key] = _build_nc(b3)
    nc = _CACHE[key]

    import ml_dtypes
    xT = host.pop("xT")
    in_maps = []
    for c in range(NCORES):
        m = dict(host)
        sl = xT[:, c * BL:(c + 1) * BL]  # [2048, BL]
        m["xTt"] = np.ascontiguousarray(
            sl.reshape(16, 128, BL).transpose(1, 0, 2).reshape(
                128, 16 * BL)).astype(ml_dtypes.bfloat16)
        in_maps.append(m)
    res = run_bass_kernel_spmd(nc, in_maps, core_ids=list(range(NCORES)))
    out = np.concatenate(
        [r["y"].reshape(BL, 1280, 3) for r in res.results], axis=0)
    return out.astype(np.float32)


if __name__ == "__main__":
    import reference as R
    inp = R.setup_inputs()
    inp = {k: np.asarray(v) for k, v in inp.items()}
    act = kernel(**inp)
    exp = np.asarray(R.reference(**inp))
    err = np.linalg.norm(act - exp) / np.linalg.norm(exp)
    print("Relative error:", err)


# revision 21
# speedup vs baseline: 1.0145x; 1.0145x over previous
"""Trainium2 Bass kernel for nn_Graph_CNN_Feat_Mesh (Chebyshev GNN decoder).

Strategy (per-core, data-parallel over batch B=256 -> 32/core):
  - All spmms are dense matmuls on the tensor engine (PE) in bf16:
      y = A + L @ (B + L @ (2C)),  A/B/C = feature-space linears of the input.
    L is densified on host; for up4-preceded layers the replication is folded
    into LU = L @ U (contracting the small pre-upsample vertex space), and the
    A/B-linear inputs are read through stride-0 broadcast APs (no copies).
  - B and A linear terms accumulate directly into the spmm PSUM.
  - Activations live in packed F-layout [(j,Fin) partitions, (b//G)*Vsp + v]
    between layers; the per-layer linear emits V-layout directly; one PE
    transpose per layer returns to F-layout, with bn_stats chunks fused in.
  - BatchNorm (training mode, global batch stats) is exact: per-core partial
    (sum, sumsq) go through an AllGather (cheaper than AllReduce) and are
    tree-reduced locally; scale/shift+relu is applied per vertex-tile chunk so
    the next layer's PE work starts immediately.
  - FC head (2048->512->5120) runs fully in bf16 with fp32 PSUM accumulation;
    weights are pre-tiled on host into single-DMA layouts.
"""

import numpy as np

B = 256
NCORES = 8
BL = B // NCORES  # 32
EPS = 1e-5

_CACHE = {}


def _split_W(W):
    W = np.asarray(W, np.float32)
    return W[:, 0::3], W[:, 1::3], W[:, 2::3]


def _dense_L(rows, cols, vals, V):
    L = np.zeros((V, V), np.float32)
    np.add.at(L, (np.asarray(rows), np.asarray(cols)), np.asarray(vals, np.float32))
    return L


def _tile_k(a, tk=128):
    """[K, N] -> [128, (K//128)*N] (k-tile-major columns)."""
    K, N = a.shape
    if K % tk:
        a = np.concatenate([a, np.zeros((tk - K % tk, N), a.dtype)], 0)
    nk = a.shape[0] // tk
    return np.ascontiguousarray(
        a.reshape(nk, tk, N).transpose(1, 0, 2).reshape(tk, nk * N))


class _LCfg:
    def __init__(self, name, Vsp, V, Fin, Fout, up4, bn):
        self.name = name
        self.Vsp = Vsp      # source vertex space of C-linear (pre-up4)
        self.V = V          # output vertex count
        self.Fin = Fin
        self.Fout = Fout
        self.G = 128 // Fin          # batches packed on partitions at input
        self.nG = BL // self.G
        self.GF = self.G * Fout      # N of one B/C/A-linear matmul
        self.Gp = 128 // Fout if Fout in (32, 64) else None
        self.nGp = BL // self.Gp if self.Gp else None
        self.up4 = up4
        self.bn = bn
        self.nVt = (V + 127) // 128
        self.nVsp = (Vsp + 127) // 128
        self.BF = BL * Fout          # free width of V-layout per vtile

    def vts(self, t):
        return min(128, self.V - t * 128)

    def sps(self, s):
        return min(128, self.Vsp - s * 128)


CFGS = [
    _LCfg("c0", 80, 320, 64, 64, True, True),
    _LCfg("c1", 320, 320, 64, 32, False, True),
    _LCfg("c2", 320, 1280, 32, 32, True, True),
    _LCfg("c3", 1280, 1280, 32, 3, False, False),
]


def _wbd(W, G, Fin, Fout, which):
    """Block-diagonal rhs weight [128, G*Fout] for the fused linear.
    which: 'A' -> W0 - W2, 'B' -> W1, 'C' -> 2*W2.  col = j*Fout + c."""
    W0, W1, W2 = _split_W(W)
    M = {"A": W0 - W2, "B": W1, "C": 2.0 * W2}[which]  # [Fout, Fin]
    out = np.zeros((128, G * Fout), np.float32)
    for j in range(G):
        out[j * Fin:(j + 1) * Fin, j * Fout:(j + 1) * Fout] = M.T
    return out


def _build_host(inputs):
    import ml_dtypes
    bf = ml_dtypes.bfloat16
    f32 = np.float32
    d = {}
    xT = np.ascontiguousarray(np.asarray(inputs["x"], f32).T)  # [2048, 256]
    d["xT"] = xT  # sliced + tiled per-core in kernel()
    d["fc1wt"] = _tile_k(np.asarray(inputs["fc1_w"], f32).T).astype(bf)
    d["fc1b"] = np.ascontiguousarray(
        np.asarray(inputs["fc1_b"], f32).reshape(4, 128).T)  # [128,4]
    # fc2: [512, 5120] -> per-mc [512, 1280] k-tiled, mc-major concat
    fc2 = np.asarray(inputs["fc2_w"], f32).T
    d["fc2wt"] = np.concatenate(
        [_tile_k(fc2[:, mc * 1280:(mc + 1) * 1280]) for mc in range(4)],
        axis=1).astype(bf)  # [128, 4*4*1280]

    L1 = _dense_L(inputs["L1_rows"], inputs["L1_cols"], inputs["L1_vals"], 320)
    L2 = _dense_L(inputs["L2_rows"], inputs["L2_cols"], inputs["L2_vals"], 1280)
    U1 = np.repeat(np.eye(80, dtype=f32), 4, axis=0)    # [320, 80]
    U2 = np.repeat(np.eye(320, dtype=f32), 4, axis=0)   # [1280, 320]
    lu0 = (L1 @ U1).T  # [80, 320]
    d["LU0"] = np.concatenate(
        [lu0, np.zeros((48, 320), f32)], 0).astype(bf)   # [128, 320]
    d["LT1t"] = _tile_k(L1.T).astype(bf)                 # [128, 3*320]
    d["LU2t"] = _tile_k((L2 @ U2).T).astype(bf)          # [128, 3*1280]
    d["LT2t"] = _tile_k(L2.T).astype(bf)                 # [128, 10*1280]

    Wn = {"c0": "cl0_w", "c1": "cl1_w", "c2": "cl2_w", "c3": "cl3_w"}
    wall = []
    for cfg in CFGS:
        W = np.asarray(inputs[Wn[cfg.name]], f32)
        for which in "ABC":
            wall.append(_wbd(W, cfg.G, cfg.Fin, cfg.Fout, which))
    d["Wall"] = np.concatenate(wall, axis=1).astype(bf)  # [128, 3*(128+64+128+12)]
    d["b3"] = np.asarray(inputs["cl3_b"], f32).copy()

    for i, (g, b) in enumerate([("bn0_g", "bn0_b"), ("bn1_g", "bn1_b"),
                                ("bn2_g", "bn2_b")]):
        F = len(np.asarray(inputs[g]))
        gbp = np.zeros((128, 2), f32)
        gbp[:F, 0] = np.asarray(inputs[g], f32)
        gbp[:F, 1] = np.asarray(inputs[b], f32)
        d[f"gbp{i}"] = gbp  # partition-major [F rows]: (gamma, beta)

    # selection matrices fold the local-mean -> global-sum/n factor:
    # sum_j mean_{p=(j,f)} * FD / n_g  with FD = nGp*V, n_g = B*V
    for F, nGp, nm in [(64, 16, "sel64"), (32, 8, "sel32")]:
        Gp = 128 // F
        sel = np.zeros((128, F), f32)
        for j in range(Gp):
            sel[j * F:(j + 1) * F] += np.eye(F, dtype=f32)
        d[nm] = sel * (nGp / float(B))
    return d


def _build_nc(b3_imm):
    import sys
    for p in ("/opt/trn_rl_repo", "/opt/trn_rl_repo/concourse"):
        if p not in sys.path:
            sys.path.insert(0, p)
    import concourse.bass as bass  # noqa
    import concourse.mybir as mybir
    import concourse.tile as tile
    from concourse import bacc
    from concourse.masks import make_identity

    f32 = mybir.dt.float32
    bf16 = mybir.dt.bfloat16
    AF = mybir.ActivationFunctionType
    ALU = mybir.AluOpType

    nc = bacc.Bacc(None, target_bir_lowering=False)

    xT = nc.dram_tensor("xTt", [128, 16 * BL], bf16, kind="ExternalInput")
    fc1wt = nc.dram_tensor("fc1wt", [128, 16 * 512], bf16, kind="ExternalInput")
    fc1b = nc.dram_tensor("fc1b", [128, 4], f32, kind="ExternalInput")
    fc2wt = nc.dram_tensor("fc2wt", [128, 16 * 1280], bf16, kind="ExternalInput")
    LU0 = nc.dram_tensor("LU0", [128, 320], bf16, kind="ExternalInput")
    LT1t = nc.dram_tensor("LT1t", [128, 3 * 320], bf16, kind="ExternalInput")
    LU2t = nc.dram_tensor("LU2t", [128, 3 * 1280], bf16, kind="ExternalInput")
    LT2t = nc.dram_tensor("LT2t", [128, 10 * 1280], bf16, kind="ExternalInput")
    WCOLS = sum(cfg.GF for cfg in CFGS) * 3
    Wallt = nc.dram_tensor("Wall", [128, WCOLS], bf16, kind="ExternalInput")
    gbs = [nc.dram_tensor(f"gbp{i}", [128, 2], f32, kind="ExternalInput")
           for i in range(3)]
    sel64 = nc.dram_tensor("sel64", [128, 64], f32, kind="ExternalInput")
    sel32 = nc.dram_tensor("sel32", [128, 32], f32, kind="ExternalInput")
    ydram = nc.dram_tensor("y", [BL, 1280 * 3], f32, kind="ExternalOutput")

    with tile.TileContext(nc) as tc:
        with (
            tc.tile_pool(name="const", bufs=1) as constp,
            tc.tile_pool(name="wpool", bufs=1) as wpool,
            tc.tile_pool(name="headp", bufs=1) as headp,
            tc.tile_pool(name="poolA", bufs=2) as poolA,
            tc.tile_pool(name="poolB", bufs=1) as poolB,
            tc.tile_pool(name="poolC", bufs=1) as poolC,
            tc.tile_pool(name="misc", bufs=1) as miscp,
            tc.tile_pool(name="outp", bufs=3) as outp,
            tc.tile_pool(name="pslin", bufs=2, space="PSUM") as pslin,
            tc.tile_pool(name="psbig", bufs=2, space="PSUM") as psbig,
            tc.tile_pool(name="pstr", bufs=2, space="PSUM") as pstr,
            tc.tile_pool(name="dram", bufs=1, space="DRAM") as dramp,
        ):
            # ---- critical-path loads first: FC head operands ----
            xT_sb = headp.tile([128, 16 * BL], bf16, tag="xT")
            nc.sync.dma_start(xT_sb[:], xT[:])
            fc1b_sb = constp.tile([128, 4], f32, tag="fc1b")
            nc.sync.dma_start(fc1b_sb[:], fc1b[:])
            fc1w_sb = headp.tile([128, 16 * 512], bf16, tag="fc1w")
            nc.sync.dma_start(fc1w_sb[:], fc1wt[:])
            fc2w_sb = headp.tile([128, 16 * 1280], bf16, tag="fc2w")
            nc.sync.dma_start(fc2w_sb[:, :8 * 1280], fc2wt[:, :8 * 1280])

            # ---- secondary loads (stream while head computes) ----
            ident_b = constp.tile([128, 128], bf16, tag="identb")
            make_identity(nc, ident_b[:])
            sel_sb = {64: constp.tile([128, 64], f32, tag="sel64", name="sel64sb"),
                      32: constp.tile([128, 32], f32, tag="sel32", name="sel32sb")}
            nc.scalar.dma_start(sel_sb[64][:], sel64[:])
            nc.scalar.dma_start(sel_sb[32][:], sel32[:])
            gb_sb = []
            for i in range(3):
                t = constp.tile([128, 2], f32, tag=f"gb{i}")
                nc.scalar.dma_start(t[:], gbs[i][:])
                gb_sb.append(t)
            eps_t = constp.tile([128, 1], f32, tag="eps")
            nc.gpsimd.memset(eps_t[:], EPS)
            ones_t = constp.tile([128, 1], f32, tag="ones")
            nc.gpsimd.memset(ones_t[:], 1.0)

            W_sb = {}
            wall_sb = wpool.tile([128, WCOLS], bf16, tag="Wall")
            nc.scalar.dma_start(wall_sb[:], Wallt[:])
            woff = 0
            for cfg in CFGS:
                for w in "ABC":
                    W_sb[f"{w}{cfg.name}"] = wall_sb[:, woff:woff + cfg.GF]
                    woff += cfg.GF

            LUT, LT = {}, {}
            t = wpool.tile([128, 320], bf16, tag="LU0")
            nc.scalar.dma_start(t[:], LU0[:])
            LUT["c0"] = t
            t = wpool.tile([128, 3 * 320], bf16, tag="LT1")
            nc.scalar.dma_start(t[:], LT1t[:])
            LT["c0"] = LT["c1"] = LUT["c1"] = t
            # big late-use loads go on the sync queue AFTER fc2w so they
            # cannot jump ahead of the head's weights on the DMA engines
            nc.sync.dma_start(fc2w_sb[:, 8 * 1280:], fc2wt[:, 8 * 1280:])
            t = wpool.tile([128, 3 * 1280], bf16, tag="LU2")
            nc.sync.dma_start(t[:], LU2t[:])
            LUT["c2"] = t
            t = wpool.tile([128, 10 * 1280], bf16, tag="LT2")
            nc.sync.dma_start(t[:], LT2t[:])
            LT["c2"] = LT["c3"] = LUT["c3"] = t

            # ================= FC head (bf16) =================
            h1T = headp.tile([128, 4 * BL], bf16, tag="h1T")
            ps1 = pslin.tile([128, 512], f32, tag="lin")
            for mt in range(4):
                for kt in range(16):
                    nc.tensor.matmul(
                        ps1[:, mt * BL:(mt + 1) * BL],
                        fc1w_sb[:, kt * 512 + mt * 128: kt * 512 + (mt + 1) * 128],
                        xT_sb[:, kt * BL:(kt + 1) * BL],
                        start=(kt == 0), stop=(kt == 15))
            for mt in range(4):
                nc.scalar.activation(
                    h1T[:, mt * BL:(mt + 1) * BL], ps1[:, mt * BL:(mt + 1) * BL],
                    AF.Relu, bias=fc1b_sb[:, mt:mt + 1])

            # fc2 in 4 column-chunks of 1280 (10 m-tiles each).
            # psum partition = (v0%2)*64+f, col = mi*BL+b ; channels c = v0*64+f.
            # dest: XF0[(b%2)*64+f, (b//2)*80 + v0],  v0 = 2*(mc*10+mi)+p0
            XF0 = poolC.tile([128, 16 * 80], bf16, tag="XF0")
            for mc in range(4):
                ps2 = pslin.tile([128, 512], f32, tag="lin")
                for mi in range(10):
                    for kt in range(4):
                        nc.tensor.matmul(
                            ps2[:, mi * BL:(mi + 1) * BL],
                            fc2w_sb[:, (mc * 4 + kt) * 1280 + mi * 128:
                                    (mc * 4 + kt) * 1280 + (mi + 1) * 128],
                            h1T[:, kt * BL:(kt + 1) * BL],
                            start=(kt == 0), stop=(kt == 3))
                src4 = ps2[:, :10 * BL].rearrange("p (i g j) -> p i g j", g=16, j=2)
                dst4 = XF0[:].rearrange("p (g u q) -> p g u q", u=40, q=2)
                for p0 in range(2):
                    for j in range(2):
                        nc.scalar.activation(
                            dst4[j * 64:(j + 1) * 64, :,
                                 mc * 10:(mc + 1) * 10, p0]
                            .rearrange("p g i -> p i g"),
                            src4[p0 * 64:(p0 + 1) * 64, :, :, j],
                            AF.Copy)

            # ================= cheby layers =================
            XF_cur = XF0
            ar_idx = 0

            for li, cfg in enumerate(CFGS):
                V, Vsp, F = cfg.V, cfg.Vsp, cfg.Fout
                BF = cfg.BF
                last = cfg.name == "c3"

                # --- replicate input for B/A linears if up4 (reuses the
                #     dead fc2w SBUF slot) ---
                if cfg.up4:
                    XFrep = headp.tile([128, cfg.nG * V], bf16, tag="fc2w")
                    s_r = XF_cur[:].rearrange("p (g w) -> p g w", w=Vsp)
                    d_r = XFrep[:].rearrange("p (g w r) -> p g w r",
                                             w=Vsp, r=4)
                    for sp in range(cfg.nVsp):
                        spz = cfg.sps(sp)
                        for r in range(4):
                            dsl = d_r[:, :, sp * 128:sp * 128 + spz, r]
                            ssl = s_r[:, :, sp * 128:sp * 128 + spz]
                            if r % 2 == 0:
                                nc.vector.tensor_copy(dsl, ssl)
                            else:
                                nc.scalar.activation(dsl, ssl, AF.Copy)
                else:
                    XFrep = XF_cur

                def rep_in(g, t, vsz, _X=XFrep, _cfg=cfg):
                    return _X[:, g * _cfg.V + t * 128:
                              g * _cfg.V + t * 128 + vsz]

                # --- C linear (in Vsp space) ---
                XC = poolC.tile([128, cfg.nVsp * BL * F], bf16, tag="XC")
                gpack = max(1, 512 // cfg.GF)
                for s in range(cfg.nVsp):
                    ssz = cfg.sps(s)
                    for g0 in range(0, cfg.nG, gpack):
                        gn = min(gpack, cfg.nG - g0)
                        pc = pslin.tile([128, 512], f32, tag="lin")
                        for gi in range(gn):
                            g = g0 + gi
                            nc.tensor.matmul(
                                pc[:ssz, gi * cfg.GF:(gi + 1) * cfg.GF],
                                XF_cur[:, g * Vsp + s * 128:
                                       g * Vsp + s * 128 + ssz],
                                W_sb[f"C{cfg.name}"],
                                start=True, stop=True)
                        nc.scalar.activation(
                            XC[:ssz, s * BL * F + g0 * cfg.GF:
                               s * BL * F + (g0 + gn) * cfg.GF],
                            pc[:ssz, :gn * cfg.GF], AF.Copy)

                # --- inner = LU @ (2C) + B ;  y = L @ inner + A ---
                Xin = poolB.tile([128, cfg.nVt * BF], bf16, tag="B")
                ytile = poolC.tile([128, cfg.nVt * BF], bf16, tag="YT")
                for phase in range(2):
                    srcL = LUT[cfg.name] if phase == 0 else LT[cfg.name]
                    nS = cfg.nVsp if phase == 0 else cfg.nVt
                    ssizes = ([cfg.sps(s) for s in range(nS)] if phase == 0
                              else [cfg.vts(s) for s in range(nS)])
                    rhs = XC if phase == 0 else Xin
                    rhs_w = BL * F if phase == 0 else BF
                    Wacc = W_sb[f"B{cfg.name}" if phase == 0 else f"A{cfg.name}"]
                    dst = Xin if phase == 0 else ytile
                    for t in range(cfg.nVt):
                        vsz = cfg.vts(t)
                        for pc0 in range(0, BF, 1024):
                            pw = min(1024, BF - pc0)
                            pi = psbig.tile([128, max(pw, 512)], f32, tag="big")
                            for nk in range(0, pw, 512):
                                n0 = pc0 + nk
                                n1 = min(n0 + 512, pc0 + pw)
                                for s in range(nS):
                                    ssz = ssizes[s]
                                    nc.tensor.matmul(
                                        pi[:vsz, n0 - pc0:n1 - pc0],
                                        srcL[:ssz, s * V + t * 128:
                                             s * V + t * 128 + vsz],
                                        rhs[:ssz, s * rhs_w + n0:
                                            s * rhs_w + n1],
                                        start=(s == 0), stop=False,
                                        skip_group_check=True)
                                for g in range(n0 // cfg.GF,
                                               (n1 + cfg.GF - 1) // cfg.GF):
                                    nc.tensor.matmul(
                                        pi[:vsz, g * cfg.GF - pc0:
                                           (g + 1) * cfg.GF - pc0],
                                        rep_in(g, t, vsz),
                                        Wacc,
                                        start=False, stop=True,
                                        skip_group_check=True)
                            if last and phase == 1:
                                # reorder (b,fo) -> (fo,b) for output staging
                                nc.vector.tensor_copy(
                                    dst[:vsz, t * BF + pc0: t * BF + pc0 + pw]
                                    .rearrange("p (c b) -> p c b", b=BL),
                                    pi[:vsz, :pw]
                                    .rearrange("p (b c) -> p c b", c=3))
                            elif phase == 0:
                                nc.scalar.activation(
                                    dst[:vsz, t * BF + pc0: t * BF + pc0 + pw],
                                    pi[:vsz, :pw], AF.Copy)
                            else:
                                nc.vector.tensor_copy(
                                    dst[:vsz, t * BF + pc0: t * BF + pc0 + pw],
                                    pi[:vsz, :pw])

                if not last:
                    # --- back-transpose to packed F-layout of next level,
                    #     with bn_stats chunks fused in ---
                    Gp, nGp = cfg.Gp, cfg.nGp
                    FD = nGp * V
                    nch = cfg.nVt * nGp
                    bnst = miscp.tile([128, nch * 6], f32, tag="bnst")
                    ch = 0
                    XFn = poolA.tile([128, nGp * V], bf16, tag="A")
                    dstv = XFn[:].rearrange("p (g v) -> p g v", v=V)
                    for t in range(cfg.nVt):
                        vsz = cfg.vts(t)
                        for q0 in range(0, nGp, 4):
                            qn = min(4, nGp - q0)
                            pt = pstr.tile([128, 512], bf16, tag="tr")
                            for qi in range(qn):
                                gp = q0 + qi
                                nc.tensor.transpose(
                                    pt[:, qi * 128: qi * 128 + vsz],
                                    ytile[:vsz, t * BF + gp * 128:
                                          t * BF + (gp + 1) * 128],
                                    ident_b[:vsz, :vsz])
                            dreg = dstv[:, q0:q0 + qn, t * 128:t * 128 + vsz]
                            nc.scalar.activation(
                                dreg,
                                pt[:].rearrange("p (q v) -> p q v", v=128)
                                [:, :qn, :vsz],
                                AF.Copy)
                            for qi in range(qn):
                                nc.vector.bn_stats(
                                    bnst[:, ch * 6:(ch + 1) * 6],
                                    dstv[:, q0 + qi, t * 128:t * 128 + vsz])
                                ch += 1

                    # --- local partials: stats_l[0, f] = local mu contrib,
                    #     stats_l[0, F+f] = local E[y^2] contrib (the 1/n_g
                    #     factor is folded into sel on host) ---
                    aggr = miscp.tile([128, 2], f32, tag="aggr")
                    nc.vector.bn_aggr(
                        aggr[:], bnst[:].rearrange("p (c s) -> p c s", s=6))
                    part = miscp.tile([128, 1], f32, tag="part")
                    nc.vector.tensor_tensor(
                        out=part[:], in0=aggr[:, 0:1], in1=aggr[:, 0:1],
                        op=ALU.mult)
                    nc.vector.tensor_tensor(
                        out=part[:], in0=part[:], in1=aggr[:, 1:2],
                        op=ALU.add)
                    pst = pslin.tile([128, 512], f32, tag="lin")
                    nc.tensor.matmul(pst[:1, :F], aggr[:, 0:1], sel_sb[F][:],
                                     start=True, stop=True)
                    nc.tensor.matmul(pst[:1, F:2 * F], part[:],
                                     sel_sb[F][:], start=True, stop=True)
                    stats_l = miscp.tile([1, 2 * F], f32, tag="statl")
                    nc.vector.tensor_copy(stats_l[:], pst[:1, :2 * F])
                    bin_ = dramp.tile([1, 2 * F], f32, tag=f"arin{ar_idx}")
                    bout = dramp.tile([NCORES, 2 * F], f32, tag=f"arout{ar_idx}")
                    nc.gpsimd.dma_start(bin_[:], stats_l[:])
                    nc.gpsimd.collective_compute(
                        "AllGather", ALU.bypass,
                        replica_groups=[list(range(NCORES))],
                        ins=[bin_.opt()], outs=[bout.opt()])
                    gsb = miscp.tile([128, 2 * F], f32, tag="gath")
                    nc.sync.dma_start(gsb[:NCORES, :], bout[:])
                    # reduce over cores via PE: mu and E[y^2] per channel,
                    # partition-major [F, 1]
                    psr = pslin.tile([128, 512], f32, tag="lin", name="psr")
                    nc.tensor.matmul(psr[:F, 0:1], gsb[:NCORES, 0:F],
                                     ones_t[:NCORES, :], start=True, stop=True)
                    nc.tensor.matmul(psr[:F, 1:2], gsb[:NCORES, F:2 * F],
                                     ones_t[:NCORES, :], start=True, stop=True)
                    stc = miscp.tile([128, 2], f32, tag=f"stc{ar_idx}")
                    tmp = miscp.tile([128, 2], f32, tag="sttmp")
                    # tmp0 = mu^2 ; tmp1 = var = E[y^2] - mu^2
                    nc.vector.tensor_tensor(out=tmp[:F, 0:1],
                                            in0=psr[:F, 0:1],
                                            in1=psr[:F, 0:1], op=ALU.mult)
                    nc.vector.tensor_tensor(out=tmp[:F, 1:2],
                                            in0=psr[:F, 1:2],
                                            in1=tmp[:F, 0:1], op=ALU.subtract)
                    nc.scalar.activation(tmp[:F, 1:2], tmp[:F, 1:2],
                                         AF.Sqrt, bias=eps_t[:F, :])
                    nc.vector.reciprocal(tmp[:F, 1:2], tmp[:F, 1:2])
                    # stc0 = s = rstd*gamma ; stc1 = t = beta - mu*s
                    nc.vector.tensor_tensor(out=stc[:F, 0:1],
                                            in0=tmp[:F, 1:2],
                                            in1=gb_sb[li][:F, 0:1], op=ALU.mult)
                    nc.vector.tensor_tensor(out=tmp[:F, 0:1],
                                            in0=psr[:F, 0:1],
                                            in1=stc[:F, 0:1], op=ALU.mult)
                    nc.vector.tensor_tensor(out=stc[:F, 1:2],
                                            in0=gb_sb[li][:F, 1:2],
                                            in1=tmp[:F, 0:1], op=ALU.subtract)
                    for j in range(1, Gp):
                        nc.vector.tensor_copy(stc[j * F:(j + 1) * F, :],
                                              stc[:F, :])
                    ar_idx += 1
                    # scale+shift+relu per vertex-tile chunk so the next
                    # layer's C-linear (which consumes s-tiles in order)
                    # starts right after chunk 0
                    for s in range(cfg.nVt):
                        vsz = cfg.vts(s)
                        nc.scalar.activation(
                            dstv[:, :, s * 128:s * 128 + vsz],
                            dstv[:, :, s * 128:s * 128 + vsz],
                            AF.Relu, scale=stc[:, 0:1], bias=stc[:, 1:2])
                    XF_cur = XFn
                else:
                    # --- stage output: ytile [v, fo*32+b] -> [b, v*3+fo] ---
                    for t in range(cfg.nVt):
                        pt = pstr.tile([128, 512], bf16, tag="tr")
                        nc.tensor.transpose(
                            pt[:96, :128],
                            ytile[:128, t * BF:(t + 1) * BF],
                            ident_b[:128, :128])
                        och = outp.tile([BL, 384], f32, tag="out")
                        for fo in range(3):
                            nc.vector.tensor_scalar_add(
                                och[:].rearrange("b (v f) -> b v f", f=3)
                                [:, :, fo],
                                pt[fo * 32:(fo + 1) * 32, :128],
                                float(b3_imm[fo]))
                        nc.sync.dma_start(
                            ydram[:, t * 384:(t + 1) * 384], och[:])

    nc.compile()
    return nc


def kernel(**inputs):
    import sys
    for p in ("/opt/trn_rl_repo", "/opt/trn_rl_repo/concourse"):
        if p not in sys.path:
            sys.path.insert(0, p)
    from concourse.bass_utils import run_bass_kernel_spmd

    host = _build_host(inputs)
    b3 = [float(v) for v in host.pop("b3")]

    key = ("nc",) + tuple(b3)
    if key not in _CACHE:
        _CACHE[key] = _build_nc(b3)
    nc = _CACHE[key]

    import ml_dtypes
    xT = host.pop("xT")
    in_maps = []
    for c in range(NCORES):
        m = dict(host)
        sl = xT[:, c * BL:(c + 1) * BL]  # [2048, BL]
        m["xTt"] = np.ascontiguousarray(
            sl.reshape(16, 128, BL).transpose(1, 0, 2).reshape(
                128, 16 * BL)).astype(ml_dtypes.bfloat16)
        in_maps.append(m)
    res = run_bass_kernel_spmd(nc, in_maps, core_ids=list(range(NCORES)))
    out = np.concatenate(
        [r["y"].reshape(BL, 1280, 3) for r in res.results], axis=0)
    return out.astype(np.float32)


if __name__ == "__main__":
    import reference as R
    inp = R.setup_inputs()
    inp = {k: np.asarray(v) for k, v in inp.items()}
    act = kernel(**inp)
    exp = np.asarray(R.reference(**inp))
    err = np.linalg.norm(act - exp) / np.linalg.norm(exp)
    print("Relative error:", err)


# revision 27
# speedup vs baseline: 1.0385x; 1.0237x over previous
"""Trainium2 Bass kernel for nn_Graph_CNN_Feat_Mesh (Chebyshev GNN decoder).

Strategy (per-core, data-parallel over batch B=256 -> 32/core):
  - All spmms are dense matmuls on the tensor engine (PE) in bf16:
      y = A + L @ (B + L @ (2C)),  A/B/C = feature-space linears of the input.
    L is densified on host; for up4-preceded layers the replication is folded
    into LU = L @ U (contracting the small pre-upsample vertex space), and the
    A/B-linear inputs are read through stride-0 broadcast APs (no copies).
  - B and A linear terms accumulate directly into the spmm PSUM.
  - Activations live in packed F-layout [(j,Fin) partitions, (b//G)*Vsp + v]
    between layers; the per-layer linear emits V-layout directly; one PE
    transpose per layer returns to F-layout, with bn_stats chunks fused in.
  - BatchNorm (training mode, global batch stats) is exact: per-core partial
    (sum, sumsq) go through an AllGather (cheaper than AllReduce) and are
    tree-reduced locally; scale/shift+relu is applied per vertex-tile chunk so
    the next layer's PE work starts immediately.
  - FC head (2048->512->5120) runs fully in bf16 with fp32 PSUM accumulation;
    weights are pre-tiled on host into single-DMA layouts.
"""

import numpy as np

B = 256
NCORES = 8
BL = B // NCORES  # 32
EPS = 1e-5

_CACHE = {}


def _split_W(W):
    W = np.asarray(W, np.float32)
    return W[:, 0::3], W[:, 1::3], W[:, 2::3]


def _dense_L(rows, cols, vals, V):
    L = np.zeros((V, V), np.float32)
    np.add.at(L, (np.asarray(rows), np.asarray(cols)), np.asarray(vals, np.float32))
    return L


def _tile_k(a, tk=128):
    """[K, N] -> [128, (K//128)*N] (k-tile-major columns)."""
    K, N = a.shape
    if K % tk:
        a = np.concatenate([a, np.zeros((tk - K % tk, N), a.dtype)], 0)
    nk = a.shape[0] // tk
    return np.ascontiguousarray(
        a.reshape(nk, tk, N).transpose(1, 0, 2).reshape(tk, nk * N))


class _LCfg:
    def __init__(self, name, Vsp, V, Fin, Fout, up4, bn):
        self.name = name
        self.Vsp = Vsp      # source vertex space of C-linear (pre-up4)
        self.V = V          # output vertex count
        self.Fin = Fin
        self.Fout = Fout
        self.G = 128 // Fin          # batches packed on partitions at input
        self.nG = BL // self.G
        self.GF = self.G * Fout      # N of one B/C/A-linear matmul
        self.Gp = 128 // Fout if Fout in (32, 64) else None
        self.nGp = BL // self.Gp if self.Gp else None
        self.up4 = up4
        self.bn = bn
        self.nVt = (V + 127) // 128
        self.nVsp = (Vsp + 127) // 128
        self.BF = BL * Fout          # free width of V-layout per vtile

    def vts(self, t):
        return min(128, self.V - t * 128)

    def sps(self, s):
        return min(128, self.Vsp - s * 128)


CFGS = [
    _LCfg("c0", 80, 320, 64, 64, True, True),
    _LCfg("c1", 320, 320, 64, 32, False, True),
    _LCfg("c2", 320, 1280, 32, 32, True, True),
    _LCfg("c3", 1280, 1280, 32, 3, False, False),
]


def _wbd(W, G, Fin, Fout, which):
    """Block-diagonal rhs weight [128, G*Fout] for the fused linear.
    which: 'A' -> W0 - W2, 'B' -> W1, 'C' -> 2*W2.  col = j*Fout + c."""
    W0, W1, W2 = _split_W(W)
    M = {"A": W0 - W2, "B": W1, "C": 2.0 * W2}[which]  # [Fout, Fin]
    out = np.zeros((128, G * Fout), np.float32)
    for j in range(G):
        out[j * Fin:(j + 1) * Fin, j * Fout:(j + 1) * Fout] = M.T
    return out


def _build_host(inputs):
    import ml_dtypes
    bf = ml_dtypes.bfloat16
    f32 = np.float32
    d = {}
    xT = np.ascontiguousarray(np.asarray(inputs["x"], f32).T)  # [2048, 256]
    d["xT"] = xT  # sliced + tiled per-core in kernel()
    d["fc1wt"] = _tile_k(np.asarray(inputs["fc1_w"], f32).T).astype(bf)
    d["fc1b"] = np.ascontiguousarray(
        np.asarray(inputs["fc1_b"], f32).reshape(4, 128).T)  # [128,4]
    # fc2: [512, 5120] -> per-mc [512, 1280] k-tiled, mc-major concat
    fc2 = np.asarray(inputs["fc2_w"], f32).T
    d["fc2wt"] = np.concatenate(
        [_tile_k(fc2[:, mc * 1280:(mc + 1) * 1280]) for mc in range(4)],
        axis=1).astype(bf)  # [128, 4*4*1280]

    L1 = _dense_L(inputs["L1_rows"], inputs["L1_cols"], inputs["L1_vals"], 320)
    L2 = _dense_L(inputs["L2_rows"], inputs["L2_cols"], inputs["L2_vals"], 1280)
    U1 = np.repeat(np.eye(80, dtype=f32), 4, axis=0)    # [320, 80]
    U2 = np.repeat(np.eye(320, dtype=f32), 4, axis=0)   # [1280, 320]
    lu0 = (L1 @ U1).T  # [80, 320]
    d["LU0"] = np.concatenate(
        [lu0, np.zeros((48, 320), f32)], 0).astype(bf)   # [128, 320]
    d["LT1t"] = _tile_k(L1.T).astype(bf)                 # [128, 3*320]
    d["LU2t"] = _tile_k((L2 @ U2).T).astype(bf)          # [128, 3*1280]
    d["LT2t"] = _tile_k(L2.T).astype(bf)                 # [128, 10*1280]

    Wn = {"c0": "cl0_w", "c1": "cl1_w", "c2": "cl2_w", "c3": "cl3_w"}
    wall = []
    for cfg in CFGS:
        W = np.asarray(inputs[Wn[cfg.name]], f32)
        for which in "ABC":
            wall.append(_wbd(W, cfg.G, cfg.Fin, cfg.Fout, which))
    d["Wall"] = np.concatenate(wall, axis=1).astype(bf)  # [128, 3*(128+64+128+12)]
    d["b3"] = np.asarray(inputs["cl3_b"], f32).copy()

    for i, (g, b) in enumerate([("bn0_g", "bn0_b"), ("bn1_g", "bn1_b"),
                                ("bn2_g", "bn2_b")]):
        F = len(np.asarray(inputs[g]))
        gbp = np.zeros((128, 2), f32)
        gbp[:F, 0] = np.asarray(inputs[g], f32)
        gbp[:F, 1] = np.asarray(inputs[b], f32)
        d[f"gbp{i}"] = gbp  # partition-major [F rows]: (gamma, beta)

    # selection matrices fold the local-mean -> global-sum/n factor:
    # sum_j mean_{p=(j,f)} * FD / n_g  with FD = nGp*V, n_g = B*V
    for F, nGp, nm in [(64, 16, "sel64"), (32, 8, "sel32")]:
        Gp = 128 // F
        sel = np.zeros((128, F), f32)
        for j in range(Gp):
            sel[j * F:(j + 1) * F] += np.eye(F, dtype=f32)
        d[nm] = sel * (nGp / float(B))
    return d


def _build_nc(b3_imm):
    import sys
    for p in ("/opt/trn_rl_repo", "/opt/trn_rl_repo/concourse"):
        if p not in sys.path:
            sys.path.insert(0, p)
    import concourse.bass as bass  # noqa
    import concourse.mybir as mybir
    import concourse.tile as tile
    from concourse import bacc
    from concourse.masks import make_identity

    f32 = mybir.dt.float32
    bf16 = mybir.dt.bfloat16
    AF = mybir.ActivationFunctionType
    ALU = mybir.AluOpType

    nc = bacc.Bacc(None, target_bir_lowering=False)

    xT = nc.dram_tensor("xTt", [128, 16 * BL], bf16, kind="ExternalInput")
    fc1wt = nc.dram_tensor("fc1wt", [128, 16 * 512], bf16, kind="ExternalInput")
    fc1b = nc.dram_tensor("fc1b", [128, 4], f32, kind="ExternalInput")
    fc2wt = nc.dram_tensor("fc2wt", [128, 16 * 1280], bf16, kind="ExternalInput")
    LU0 = nc.dram_tensor("LU0", [128, 320], bf16, kind="ExternalInput")
    LT1t = nc.dram_tensor("LT1t", [128, 3 * 320], bf16, kind="ExternalInput")
    LU2t = nc.dram_tensor("LU2t", [128, 3 * 1280], bf16, kind="ExternalInput")
    LT2t = nc.dram_tensor("LT2t", [128, 10 * 1280], bf16, kind="ExternalInput")
    WCOLS = sum(cfg.GF for cfg in CFGS) * 3
    Wallt = nc.dram_tensor("Wall", [128, WCOLS], bf16, kind="ExternalInput")
    gbs = [nc.dram_tensor(f"gbp{i}", [128, 2], f32, kind="ExternalInput")
           for i in range(3)]
    sel64 = nc.dram_tensor("sel64", [128, 64], f32, kind="ExternalInput")
    sel32 = nc.dram_tensor("sel32", [128, 32], f32, kind="ExternalInput")
    ydram = nc.dram_tensor("y", [BL, 1280 * 3], f32, kind="ExternalOutput")

    with tile.TileContext(nc) as tc:
        with (
            tc.tile_pool(name="const", bufs=1) as constp,
            tc.tile_pool(name="wpool", bufs=1) as wpool,
            tc.tile_pool(name="headp", bufs=1) as headp,
            tc.tile_pool(name="poolA", bufs=2) as poolA,
            tc.tile_pool(name="poolB", bufs=1) as poolB,
            tc.tile_pool(name="poolC", bufs=1) as poolC,
            tc.tile_pool(name="misc", bufs=1) as miscp,
            tc.tile_pool(name="outp", bufs=3) as outp,
            tc.tile_pool(name="pslin", bufs=2, space="PSUM") as pslin,
            tc.tile_pool(name="psbig", bufs=2, space="PSUM") as psbig,
            tc.tile_pool(name="pstr", bufs=2, space="PSUM") as pstr,
            tc.tile_pool(name="dram", bufs=1, space="DRAM") as dramp,
        ):
            # ---- critical-path loads first: FC head operands, chunked so
            #      the PE can chase the DMA stream ----
            xT_sb = headp.tile([128, 16 * BL], bf16, tag="xT")
            nc.sync.dma_start(xT_sb[:], xT[:])
            fc1b_sb = constp.tile([128, 4], f32, tag="fc1b")
            nc.sync.dma_start(fc1b_sb[:], fc1b[:])
            fc1w_sb = headp.tile([128, 16 * 512], bf16, tag="fc1w")
            for q in range(4):
                nc.sync.dma_start(
                    fc1w_sb[:, q * 4 * 512:(q + 1) * 4 * 512],
                    fc1wt[:, q * 4 * 512:(q + 1) * 4 * 512])
            fc2w_sb = headp.tile([128, 16 * 1280], bf16, tag="fc2w")
            for q in range(2):
                nc.sync.dma_start(
                    fc2w_sb[:, q * 4 * 1280:(q + 1) * 4 * 1280],
                    fc2wt[:, q * 4 * 1280:(q + 1) * 4 * 1280])

            # ---- secondary loads (stream while head computes) ----
            ident_b = constp.tile([128, 128], bf16, tag="identb")
            make_identity(nc, ident_b[:])
            sel_sb = {64: constp.tile([128, 64], f32, tag="sel64", name="sel64sb"),
                      32: constp.tile([128, 32], f32, tag="sel32", name="sel32sb")}
            nc.scalar.dma_start(sel_sb[64][:], sel64[:])
            nc.scalar.dma_start(sel_sb[32][:], sel32[:])
            gb_sb = []
            for i in range(3):
                t = constp.tile([128, 2], f32, tag=f"gb{i}")
                nc.scalar.dma_start(t[:], gbs[i][:])
                gb_sb.append(t)
            eps_t = constp.tile([128, 1], f32, tag="eps")
            nc.gpsimd.memset(eps_t[:], EPS)
            ones_t = constp.tile([128, 1], f32, tag="ones")
            nc.gpsimd.memset(ones_t[:], 1.0)

            W_sb = {}
            wall_sb = wpool.tile([128, WCOLS], bf16, tag="Wall")
            nc.scalar.dma_start(wall_sb[:], Wallt[:])
            woff = 0
            for cfg in CFGS:
                for w in "ABC":
                    W_sb[f"{w}{cfg.name}"] = wall_sb[:, woff:woff + cfg.GF]
                    woff += cfg.GF

            LUT, LT = {}, {}
            t = wpool.tile([128, 320], bf16, tag="LU0")
            nc.scalar.dma_start(t[:], LU0[:])
            LUT["c0"] = t
            t = wpool.tile([128, 3 * 320], bf16, tag="LT1")
            nc.scalar.dma_start(t[:], LT1t[:])
            LT["c0"] = LT["c1"] = LUT["c1"] = t
            # big late-use loads go on the sync queue AFTER fc2w so they
            # cannot jump ahead of the head's weights on the DMA engines
            for q in range(2, 4):
                nc.sync.dma_start(
                    fc2w_sb[:, q * 4 * 1280:(q + 1) * 4 * 1280],
                    fc2wt[:, q * 4 * 1280:(q + 1) * 4 * 1280])
            t = wpool.tile([128, 3 * 1280], bf16, tag="LU2")
            nc.sync.dma_start(t[:], LU2t[:])
            LUT["c2"] = t
            t = wpool.tile([128, 10 * 1280], bf16, tag="LT2")
            for q in range(2):
                nc.sync.dma_start(t[:, q * 5 * 1280:(q + 1) * 5 * 1280],
                                  LT2t[:, q * 5 * 1280:(q + 1) * 5 * 1280])
            LT["c2"] = LT["c3"] = LUT["c3"] = t

            # ================= FC head (bf16) =================
            h1T = headp.tile([128, 4 * BL], bf16, tag="h1T")
            ps1 = pslin.tile([128, 512], f32, tag="lin")
            for mt in range(4):
                for kt in range(16):
                    nc.tensor.matmul(
                        ps1[:, mt * BL:(mt + 1) * BL],
                        fc1w_sb[:, kt * 512 + mt * 128: kt * 512 + (mt + 1) * 128],
                        xT_sb[:, kt * BL:(kt + 1) * BL],
                        start=(kt == 0), stop=(kt == 15))
            for mt in range(4):
                nc.scalar.activation(
                    h1T[:, mt * BL:(mt + 1) * BL], ps1[:, mt * BL:(mt + 1) * BL],
                    AF.Relu, bias=fc1b_sb[:, mt:mt + 1])

            # fc2 in 4 column-chunks of 1280 (10 m-tiles each).
            # psum partition = (v0%2)*64+f, col = mi*BL+b ; channels c = v0*64+f.
            # dest: XF0[(b%2)*64+f, (b//2)*80 + v0],  v0 = 2*(mc*10+mi)+p0
            XF0 = poolC.tile([128, 16 * 80], bf16, tag="XF0")
            for mc in range(4):
                ps2 = pslin.tile([128, 512], f32, tag="lin")
                for mi in range(10):
                    for kt in range(4):
                        nc.tensor.matmul(
                            ps2[:, mi * BL:(mi + 1) * BL],
                            fc2w_sb[:, (mc * 4 + kt) * 1280 + mi * 128:
                                    (mc * 4 + kt) * 1280 + (mi + 1) * 128],
                            h1T[:, kt * BL:(kt + 1) * BL],
                            start=(kt == 0), stop=(kt == 3))
                src4 = ps2[:, :10 * BL].rearrange("p (i g j) -> p i g j", g=16, j=2)
                dst4 = XF0[:].rearrange("p (g u q) -> p g u q", u=40, q=2)
                for p0 in range(2):
                    for j in range(2):
                        nc.scalar.activation(
                            dst4[j * 64:(j + 1) * 64, :,
                                 mc * 10:(mc + 1) * 10, p0]
                            .rearrange("p g i -> p i g"),
                            src4[p0 * 64:(p0 + 1) * 64, :, :, j],
                            AF.Copy)

            # ================= cheby layers =================
            XF_cur = XF0
            ar_idx = 0

            for li, cfg in enumerate(CFGS):
                V, Vsp, F = cfg.V, cfg.Vsp, cfg.Fout
                BF = cfg.BF
                last = cfg.name == "c3"

                # --- replicate input for B/A linears if up4 (reuses the
                #     dead fc2w SBUF slot) ---
                if cfg.up4:
                    XFrep = headp.tile([128, cfg.nG * V], bf16, tag="fc2w")
                    s_r = XF_cur[:].rearrange("p (g w) -> p g w", w=Vsp)
                    d_r = XFrep[:].rearrange("p (g w r) -> p g w r",
                                             w=Vsp, r=4)
                    for sp in range(cfg.nVsp):
                        spz = cfg.sps(sp)
                        for r in range(4):
                            dsl = d_r[:, :, sp * 128:sp * 128 + spz, r]
                            ssl = s_r[:, :, sp * 128:sp * 128 + spz]
                            if r % 2 == 0:
                                nc.vector.tensor_copy(dsl, ssl)
                            else:
                                nc.scalar.activation(dsl, ssl, AF.Copy)
                else:
                    XFrep = XF_cur

                def rep_in(g, t, vsz, _X=XFrep, _cfg=cfg):
                    return _X[:, g * _cfg.V + t * 128:
                              g * _cfg.V + t * 128 + vsz]

                # --- C linear (in Vsp space) ---
                XC = poolC.tile([128, cfg.nVsp * BL * F], bf16, tag="XC")
                gpack = max(1, 512 // cfg.GF)
                for s in range(cfg.nVsp):
                    ssz = cfg.sps(s)
                    for g0 in range(0, cfg.nG, gpack):
                        gn = min(gpack, cfg.nG - g0)
                        pc = pslin.tile([128, 512], f32, tag="lin")
                        for gi in range(gn):
                            g = g0 + gi
                            nc.tensor.matmul(
                                pc[:ssz, gi * cfg.GF:(gi + 1) * cfg.GF],
                                XF_cur[:, g * Vsp + s * 128:
                                       g * Vsp + s * 128 + ssz],
                                W_sb[f"C{cfg.name}"],
                                start=True, stop=True)
                        xsl = XC[:ssz, s * BL * F + g0 * cfg.GF:
                                 s * BL * F + (g0 + gn) * cfg.GF]
                        if s % 2 == 0:
                            nc.scalar.activation(
                                xsl, pc[:ssz, :gn * cfg.GF], AF.Copy)
                        else:
                            nc.vector.tensor_copy(xsl, pc[:ssz, :gn * cfg.GF])

                # --- inner = LU @ (2C) + B ;  y = L @ inner + A ---
                Xin = poolB.tile([128, cfg.nVt * BF], bf16, tag="B")
                ytile = poolC.tile([128, cfg.nVt * BF], bf16, tag="YT")
                for phase in range(2):
                    srcL = LUT[cfg.name] if phase == 0 else LT[cfg.name]
                    nS = cfg.nVsp if phase == 0 else cfg.nVt
                    ssizes = ([cfg.sps(s) for s in range(nS)] if phase == 0
                              else [cfg.vts(s) for s in range(nS)])
                    rhs = XC if phase == 0 else Xin
                    rhs_w = BL * F if phase == 0 else BF
                    Wacc = W_sb[f"B{cfg.name}" if phase == 0 else f"A{cfg.name}"]
                    dst = Xin if phase == 0 else ytile
                    for t in range(cfg.nVt):
                        vsz = cfg.vts(t)
                        for pc0 in range(0, BF, 1024):
                            pw = min(1024, BF - pc0)
                            pi = psbig.tile([128, max(pw, 512)], f32, tag="big")
                            for nk in range(0, pw, 512):
                                n0 = pc0 + nk
                                n1 = min(n0 + 512, pc0 + pw)
                                for s in range(nS):
                                    ssz = ssizes[s]
                                    nc.tensor.matmul(
                                        pi[:vsz, n0 - pc0:n1 - pc0],
                                        srcL[:ssz, s * V + t * 128:
                                             s * V + t * 128 + vsz],
                                        rhs[:ssz, s * rhs_w + n0:
                                            s * rhs_w + n1],
                                        start=(s == 0), stop=False,
                                        skip_group_check=True)
                                for g in range(n0 // cfg.GF,
                                               (n1 + cfg.GF - 1) // cfg.GF):
                                    nc.tensor.matmul(
                                        pi[:vsz, g * cfg.GF - pc0:
                                           (g + 1) * cfg.GF - pc0],
                                        rep_in(g, t, vsz),
                                        Wacc,
                                        start=False, stop=True,
                                        skip_group_check=True)
                            if last and phase == 1:
                                # reorder (b,fo) -> (fo,b) for output staging
                                nc.vector.tensor_copy(
                                    dst[:vsz, t * BF + pc0: t * BF + pc0 + pw]
                                    .rearrange("p (c b) -> p c b", b=BL),
                                    pi[:vsz, :pw]
                                    .rearrange("p (b c) -> p c b", c=3))
                            else:
                                dsl = dst[:vsz,
                                          t * BF + pc0: t * BF + pc0 + pw]
                                if (t + phase) % 2 == 0:
                                    nc.scalar.activation(
                                        dsl, pi[:vsz, :pw], AF.Copy)
                                else:
                                    nc.vector.tensor_copy(dsl, pi[:vsz, :pw])

                if not last:
                    # --- back-transpose to packed F-layout of next level,
                    #     with bn_stats chunks fused in ---
                    Gp, nGp = cfg.Gp, cfg.nGp
                    FD = nGp * V
                    vchunks = [(a, min(a + 512, V)) for a in range(0, V, 512)]
                    nch = (nGp * len(vchunks) if V > 512 else cfg.nVt * nGp)
                    bnst = miscp.tile([128, nch * 6], f32, tag="bnst")
                    ch = 0
                    XFn = poolA.tile([128, nGp * V], bf16, tag="A")
                    dstv = XFn[:].rearrange("p (g v) -> p g v", v=V)
                    for t in range(cfg.nVt):
                        vsz = cfg.vts(t)
                        for q0 in range(0, nGp, 8):
                            qn = min(8, nGp - q0)
                            pt = pstr.tile([128, 1024], bf16, tag="tr")
                            for qi in range(qn):
                                gp = q0 + qi
                                nc.tensor.transpose(
                                    pt[:, qi * 128: qi * 128 + vsz],
                                    ytile[:vsz, t * BF + gp * 128:
                                          t * BF + (gp + 1) * 128],
                                    ident_b[:vsz, :vsz])
                            dreg = dstv[:, q0:q0 + qn, t * 128:t * 128 + vsz]
                            psrc = (pt[:].rearrange("p (q v) -> p q v", v=128)
                                    [:, :qn, :vsz])
                            if (t + q0) % 2 == 0:
                                nc.scalar.activation(dreg, psrc, AF.Copy)
                            else:
                                nc.vector.tensor_copy(dreg, psrc)
                            if V <= 512:
                                for qi in range(qn):
                                    nc.vector.bn_stats(
                                        bnst[:, ch * 6:(ch + 1) * 6],
                                        dstv[:, q0 + qi,
                                             t * 128:t * 128 + vsz])
                                    ch += 1
                        if V > 512:
                            # stats over contiguous 512-wide v-chunks per g,
                            # emitted as soon as the covering tiles land
                            for ci, (a, b_) in enumerate(vchunks):
                                if t * 128 + vsz == b_:
                                    for g in range(nGp):
                                        nc.vector.bn_stats(
                                            bnst[:, ch * 6:(ch + 1) * 6],
                                            dstv[:, g, a:b_])
                                        ch += 1

                    # --- local partials: stats_l[0, f] = local mu contrib,
                    #     stats_l[0, F+f] = local E[y^2] contrib (the 1/n_g
                    #     factor is folded into sel on host) ---
                    aggr = miscp.tile([128, 2], f32, tag="aggr")
                    nc.vector.bn_aggr(
                        aggr[:], bnst[:].rearrange("p (c s) -> p c s", s=6))
                    part = miscp.tile([128, 1], f32, tag="part")
                    nc.vector.tensor_tensor(
                        out=part[:], in0=aggr[:, 0:1], in1=aggr[:, 0:1],
                        op=ALU.mult)
                    nc.vector.tensor_tensor(
                        out=part[:], in0=part[:], in1=aggr[:, 1:2],
                        op=ALU.add)
                    pst = pslin.tile([128, 512], f32, tag="lin")
                    nc.tensor.matmul(pst[:1, :F], aggr[:, 0:1], sel_sb[F][:],
                                     start=True, stop=True)
                    nc.tensor.matmul(pst[:1, F:2 * F], part[:],
                                     sel_sb[F][:], start=True, stop=True)
                    stats_l = miscp.tile([1, 2 * F], f32, tag="statl")
                    nc.vector.tensor_copy(stats_l[:], pst[:1, :2 * F])
                    bin_ = dramp.tile([1, 2 * F], f32, tag=f"arin{ar_idx}")
                    bout = dramp.tile([NCORES, 2 * F], f32, tag=f"arout{ar_idx}")
                    nc.gpsimd.dma_start(bin_[:], stats_l[:])
                    nc.gpsimd.collective_compute(
                        "AllGather", ALU.bypass,
                        replica_groups=[list(range(NCORES))],
                        ins=[bin_.opt()], outs=[bout.opt()])
                    gsb = miscp.tile([128, 2 * F], f32, tag="gath")
                    nc.sync.dma_start(gsb[:NCORES, :], bout[:])
                    # reduce over cores via PE: mu and E[y^2] per channel,
                    # partition-major [F, 1]
                    psr = pslin.tile([128, 512], f32, tag="lin", name="psr")
                    nc.tensor.matmul(psr[:F, 0:1], gsb[:NCORES, 0:F],
                                     ones_t[:NCORES, :], start=True, stop=True)
                    nc.tensor.matmul(psr[:F, 1:2], gsb[:NCORES, F:2 * F],
                                     ones_t[:NCORES, :], start=True, stop=True)
                    stc = miscp.tile([128, 2], f32, tag=f"stc{ar_idx}")
                    tmp = miscp.tile([128, 2], f32, tag="sttmp")
                    # tmp0 = mu^2 ; tmp1 = var = E[y^2] - mu^2
                    nc.vector.tensor_tensor(out=tmp[:F, 0:1],
                                            in0=psr[:F, 0:1],
                                            in1=psr[:F, 0:1], op=ALU.mult)
                    nc.vector.tensor_tensor(out=tmp[:F, 1:2],
                                            in0=psr[:F, 1:2],
                                            in1=tmp[:F, 0:1], op=ALU.subtract)
                    nc.scalar.activation(tmp[:F, 1:2], tmp[:F, 1:2],
                                         AF.Sqrt, bias=eps_t[:F, :])
                    nc.vector.reciprocal(tmp[:F, 1:2], tmp[:F, 1:2])
                    # stc0 = s = rstd*gamma ; stc1 = t = beta - mu*s
                    nc.vector.tensor_tensor(out=stc[:F, 0:1],
                                            in0=tmp[:F, 1:2],
                                            in1=gb_sb[li][:F, 0:1], op=ALU.mult)
                    nc.vector.tensor_tensor(out=tmp[:F, 0:1],
                                            in0=psr[:F, 0:1],
                                            in1=stc[:F, 0:1], op=ALU.mult)
                    nc.vector.tensor_tensor(out=stc[:F, 1:2],
                                            in0=gb_sb[li][:F, 1:2],
                                            in1=tmp[:F, 0:1], op=ALU.subtract)
                    for j in range(1, Gp):
                        nc.vector.tensor_copy(stc[j * F:(j + 1) * F, :],
                                              stc[:F, :])
                    ar_idx += 1
                    # scale+shift+relu per vertex-tile chunk so the next
                    # layer's C-linear (which consumes s-tiles in order)
                    # starts right after chunk 0
                    for s in range(cfg.nVt):
                        vsz = cfg.vts(s)
                        reg = dstv[:, :, s * 128:s * 128 + vsz]
                        if s % 2 == 0:
                            nc.scalar.activation(
                                reg, reg, AF.Relu,
                                scale=stc[:, 0:1], bias=stc[:, 1:2])
                        else:
                            nc.vector.tensor_scalar(
                                reg, reg, stc[:, 0:1], stc[:, 1:2],
                                ALU.mult, ALU.add)
                            nc.vector.tensor_scalar_max(reg, reg, 0.0)
                    XF_cur = XFn
                else:
                    # --- stage output: ytile [v, fo*32+b] -> [b, v*3+fo] ---
                    for t in range(cfg.nVt):
                        pt = pstr.tile([128, 512], bf16, tag="tr")
                        nc.tensor.transpose(
                            pt[:96, :128],
                            ytile[:128, t * BF:(t + 1) * BF],
                            ident_b[:128, :128])
                        och = outp.tile([BL, 384], f32, tag="out")
                        for fo in range(3):
                            nc.vector.tensor_scalar_add(
                                och[:].rearrange("b (v f) -> b v f", f=3)
                                [:, :, fo],
                                pt[fo * 32:(fo + 1) * 32, :128],
                                float(b3_imm[fo]))
                        nc.sync.dma_start(
                            ydram[:, t * 384:(t + 1) * 384], och[:])

    nc.compile()
    return nc


def kernel(**inputs):
    import sys
    for p in ("/opt/trn_rl_repo", "/opt/trn_rl_repo/concourse"):
        if p not in sys.path:
            sys.path.insert(0, p)
    from concourse.bass_utils import run_bass_kernel_spmd

    host = _build_host(inputs)
    b3 = [float(v) for v in host.pop("b3")]

    key = ("nc",) + tuple(b3)
    if key not in _CACHE:
        _CACHE[key] = _build_nc(b3)
    nc = _CACHE[key]

    import ml_dtypes
    xT = host.pop("xT")
    in_maps = []
    for c in range(NCORES):
        m = dict(host)
        sl = xT[:, c * BL:(c + 1) * BL]  # [2048, BL]
        m["xTt"] = np.ascontiguousarray(
            sl.reshape(16, 128, BL).transpose(1, 0, 2).reshape(
                128, 16 * BL)).astype(ml_dtypes.bfloat16)
        in_maps.append(m)
    res = run_bass_kernel_spmd(nc, in_maps, core_ids=list(range(NCORES)))
    out = np.concatenate(
        [r["y"].reshape(BL, 1280, 3) for r in res.results], axis=0)
    return out.astype(np.float32)


if __name__ == "__main__":
    import reference as R
    inp = R.setup_inputs()
    inp = {k: np.asarray(v) for k, v in inp.items()}
    act = kernel(**inp)
    exp = np.asarray(R.reference(**inp))
    err = np.linalg.norm(act - exp) / np.linalg.norm(exp)
    print("Relative error:", err)


# revision 39
# speedup vs baseline: 1.0940x; 1.0535x over previous
"""Trainium2 Bass kernel for nn_Graph_CNN_Feat_Mesh (Chebyshev GNN decoder).

Strategy (per-core, data-parallel over batch B=256 -> 32/core):
  - All spmms are dense matmuls on the tensor engine (PE) in bf16:
      y = A + L @ (B + L @ (2C)),  A/B/C = feature-space linears of the input.
    L is densified on host; for up4-preceded layers the replication is folded
    into LU = L @ U (contracting the small pre-upsample vertex space), and the
    A/B-linear inputs are read through stride-0 broadcast APs (no copies).
  - B and A linear terms accumulate directly into the spmm PSUM.
  - Activations live in packed F-layout [(j,Fin) partitions, (b//G)*Vsp + v]
    between layers; the per-layer linear emits V-layout directly; one PE
    transpose per layer returns to F-layout, with bn_stats chunks fused in.
  - BatchNorm (training mode, global batch stats) is exact: per-core partial
    (sum, sumsq) go through an AllGather (cheaper than AllReduce) and are
    tree-reduced locally; scale/shift+relu is applied per vertex-tile chunk so
    the next layer's PE work starts immediately.
  - FC head (2048->512->5120) runs fully in bf16 with fp32 PSUM accumulation;
    weights are pre-tiled on host into single-DMA layouts.
"""

import numpy as np

B = 256
NCORES = 8
BL = B // NCORES  # 32
EPS = 1e-5

_CACHE = {}


def _split_W(W):
    W = np.asarray(W, np.float32)
    return W[:, 0::3], W[:, 1::3], W[:, 2::3]


def _dense_L(rows, cols, vals, V):
    L = np.zeros((V, V), np.float32)
    np.add.at(L, (np.asarray(rows), np.asarray(cols)), np.asarray(vals, np.float32))
    return L


def _tile_k(a, tk=128):
    """[K, N] -> [128, (K//128)*N] (k-tile-major columns)."""
    K, N = a.shape
    if K % tk:
        a = np.concatenate([a, np.zeros((tk - K % tk, N), a.dtype)], 0)
    nk = a.shape[0] // tk
    return np.ascontiguousarray(
        a.reshape(nk, tk, N).transpose(1, 0, 2).reshape(tk, nk * N))


class _LCfg:
    def __init__(self, name, Vsp, V, Fin, Fout, up4, bn):
        self.name = name
        self.Vsp = Vsp      # source vertex space of C-linear (pre-up4)
        self.V = V          # output vertex count
        self.Fin = Fin
        self.Fout = Fout
        self.G = 128 // Fin          # batches packed on partitions at input
        self.nG = BL // self.G
        self.GF = self.G * Fout      # N of one B/C/A-linear matmul
        self.Gp = 128 // Fout if Fout in (32, 64) else None
        self.nGp = BL // self.Gp if self.Gp else None
        self.up4 = up4
        self.bn = bn
        self.nVt = (V + 127) // 128
        self.nVsp = (Vsp + 127) // 128
        self.BF = BL * Fout          # free width of V-layout per vtile

    def vts(self, t):
        return min(128, self.V - t * 128)

    def sps(self, s):
        return min(128, self.Vsp - s * 128)


CFGS = [
    _LCfg("c0", 80, 320, 64, 64, True, True),
    _LCfg("c1", 320, 320, 64, 32, False, True),
    _LCfg("c2", 320, 1280, 32, 32, True, True),
    _LCfg("c3", 1280, 1280, 32, 3, False, False),
]


def _wbd(W, G, Fin, Fout, which):
    """Block-diagonal rhs weight [128, G*Fout] for the fused linear.
    which: 'A' -> W0 - W2, 'B' -> W1, 'C' -> 2*W2.  col = j*Fout + c."""
    W0, W1, W2 = _split_W(W)
    M = {"A": W0 - W2, "B": W1, "C": 2.0 * W2}[which]  # [Fout, Fin]
    out = np.zeros((128, G * Fout), np.float32)
    for j in range(G):
        out[j * Fin:(j + 1) * Fin, j * Fout:(j + 1) * Fout] = M.T
    return out


def _build_host(inputs):
    import ml_dtypes
    bf = ml_dtypes.bfloat16
    f32 = np.float32
    d = {}
    xT = np.ascontiguousarray(np.asarray(inputs["x"], f32).T)  # [2048, 256]
    d["xT"] = xT  # sliced + tiled per-core in kernel()
    d["fc1wt"] = _tile_k(np.asarray(inputs["fc1_w"], f32).T).astype(bf)
    d["fc1b"] = np.ascontiguousarray(
        np.asarray(inputs["fc1_b"], f32).reshape(4, 128).T)  # [128,4]
    # fc2: [512, 5120] -> per-mc [512, 1280] k-tiled, mc-major concat
    fc2 = np.asarray(inputs["fc2_w"], f32).T
    d["fc2wt"] = np.concatenate(
        [_tile_k(fc2[:, mc * 1280:(mc + 1) * 1280]) for mc in range(4)],
        axis=1).astype(bf)  # [128, 4*4*1280]

    L1 = _dense_L(inputs["L1_rows"], inputs["L1_cols"], inputs["L1_vals"], 320)
    L2 = _dense_L(inputs["L2_rows"], inputs["L2_cols"], inputs["L2_vals"], 1280)
    U1 = np.repeat(np.eye(80, dtype=f32), 4, axis=0)    # [320, 80]
    U2 = np.repeat(np.eye(320, dtype=f32), 4, axis=0)   # [1280, 320]
    lu0 = (L1 @ U1).T  # [80, 320]
    d["LU0"] = np.concatenate(
        [lu0, np.zeros((48, 320), f32)], 0).astype(bf)   # [128, 320]
    d["LT1t"] = _tile_k(L1.T).astype(bf)                 # [128, 3*320]
    d["LU2t"] = _tile_k((L2 @ U2).T).astype(bf)          # [128, 3*1280]
    d["LT2t"] = _tile_k(L2.T).astype(bf)                 # [128, 10*1280]

    Wn = {"c0": "cl0_w", "c1": "cl1_w", "c2": "cl2_w", "c3": "cl3_w"}
    wall = []
    for cfg in CFGS:
        W = np.asarray(inputs[Wn[cfg.name]], f32)
        for which in "ABC":
            wall.append(_wbd(W, cfg.G, cfg.Fin, cfg.Fout, which))
    d["Wall"] = np.concatenate(wall, axis=1).astype(bf)  # [128, 3*(128+64+128+12)]
    d["b3"] = np.asarray(inputs["cl3_b"], f32).copy()

    for i, (g, b) in enumerate([("bn0_g", "bn0_b"), ("bn1_g", "bn1_b"),
                                ("bn2_g", "bn2_b")]):
        F = len(np.asarray(inputs[g]))
        gbp = np.zeros((128, 2), f32)
        gbp[:F, 0] = np.asarray(inputs[g], f32)
        gbp[:F, 1] = np.asarray(inputs[b], f32)
        d[f"gbp{i}"] = gbp  # partition-major [F rows]: (gamma, beta)

    # selection matrices fold the local-mean -> global-sum/n factor:
    # sum_j mean_{p=(j,f)} * FD / n_g  with FD = nGp*V, n_g = B*V
    for F, nGp, nm in [(64, 16, "sel64"), (32, 8, "sel32")]:
        Gp = 128 // F
        sel = np.zeros((128, F), f32)
        for j in range(Gp):
            sel[j * F:(j + 1) * F] += np.eye(F, dtype=f32)
        d[nm] = sel * (nGp / float(B))
    return d


def _build_nc(b3_imm):
    import sys
    for p in ("/opt/trn_rl_repo", "/opt/trn_rl_repo/concourse"):
        if p not in sys.path:
            sys.path.insert(0, p)
    import concourse.bass as bass  # noqa
    import concourse.mybir as mybir
    import concourse.tile as tile
    from concourse import bacc
    from concourse.masks import make_identity

    f32 = mybir.dt.float32
    bf16 = mybir.dt.bfloat16
    AF = mybir.ActivationFunctionType
    ALU = mybir.AluOpType

    nc = bacc.Bacc(None, target_bir_lowering=False)

    xT = nc.dram_tensor("xTt", [128, 16 * BL], bf16, kind="ExternalInput")
    fc1wt = nc.dram_tensor("fc1wt", [128, 16 * 512], bf16, kind="ExternalInput")
    fc1b = nc.dram_tensor("fc1b", [128, 4], f32, kind="ExternalInput")
    fc2wt = nc.dram_tensor("fc2wt", [128, 16 * 1280], bf16, kind="ExternalInput")
    LU0 = nc.dram_tensor("LU0", [128, 320], bf16, kind="ExternalInput")
    LT1t = nc.dram_tensor("LT1t", [128, 3 * 320], bf16, kind="ExternalInput")
    LU2t = nc.dram_tensor("LU2t", [128, 3 * 1280], bf16, kind="ExternalInput")
    LT2t = nc.dram_tensor("LT2t", [128, 10 * 1280], bf16, kind="ExternalInput")
    WCOLS = sum(cfg.GF for cfg in CFGS) * 3
    Wallt = nc.dram_tensor("Wall", [128, WCOLS], bf16, kind="ExternalInput")
    gbs = [nc.dram_tensor(f"gbp{i}", [128, 2], f32, kind="ExternalInput")
           for i in range(3)]
    sel64 = nc.dram_tensor("sel64", [128, 64], f32, kind="ExternalInput")
    sel32 = nc.dram_tensor("sel32", [128, 32], f32, kind="ExternalInput")
    ydram = nc.dram_tensor("y", [BL, 1280 * 3], f32, kind="ExternalOutput")

    with tile.TileContext(nc) as tc:
        with (
            tc.tile_pool(name="const", bufs=1) as constp,
            tc.tile_pool(name="wpool", bufs=1) as wpool,
            tc.tile_pool(name="headp", bufs=1) as headp,
            tc.tile_pool(name="poolA", bufs=2) as poolA,
            tc.tile_pool(name="poolB", bufs=1) as poolB,
            tc.tile_pool(name="poolC", bufs=1) as poolC,
            tc.tile_pool(name="misc", bufs=1) as miscp,
            tc.tile_pool(name="outp", bufs=2) as outp,
            tc.tile_pool(name="pslin", bufs=2, space="PSUM") as pslin,
            tc.tile_pool(name="psbig", bufs=2, space="PSUM") as psbig,
            tc.tile_pool(name="pstr", bufs=2, space="PSUM") as pstr,
            tc.tile_pool(name="dram", bufs=1, space="DRAM") as dramp,
        ):
            # ---- critical-path loads first: FC head operands, chunked so
            #      the PE can chase the DMA stream ----
            xT_sb = headp.tile([128, 16 * BL], bf16, tag="xT")
            nc.sync.dma_start(xT_sb[:], xT[:])
            fc1b_sb = constp.tile([128, 4], f32, tag="fc1b")
            nc.sync.dma_start(fc1b_sb[:], fc1b[:])
            fc1w_sb = headp.tile([128, 16 * 512], bf16, tag="fc1w")
            for q in range(4):
                nc.sync.dma_start(
                    fc1w_sb[:, q * 4 * 512:(q + 1) * 4 * 512],
                    fc1wt[:, q * 4 * 512:(q + 1) * 4 * 512])
            fc2w_sb = headp.tile([128, 16 * 1280], bf16, tag="fc2w")
            for q in range(2):
                nc.sync.dma_start(
                    fc2w_sb[:, q * 4 * 1280:(q + 1) * 4 * 1280],
                    fc2wt[:, q * 4 * 1280:(q + 1) * 4 * 1280])

            # ---- secondary loads (stream while head computes) ----
            ident_b = constp.tile([128, 128], bf16, tag="identb")
            make_identity(nc, ident_b[:])
            sel_sb = {64: constp.tile([128, 64], f32, tag="sel64", name="sel64sb"),
                      32: constp.tile([128, 32], f32, tag="sel32", name="sel32sb")}
            nc.sync.dma_start(sel_sb[64][:], sel64[:])
            nc.sync.dma_start(sel_sb[32][:], sel32[:])
            gb_sb = []
            for i in range(3):
                t = constp.tile([128, 2], f32, tag=f"gb{i}")
                nc.sync.dma_start(t[:], gbs[i][:])
                gb_sb.append(t)
            eps_t = constp.tile([128, 1], f32, tag="eps")
            nc.gpsimd.memset(eps_t[:], EPS)
            ones_t = constp.tile([128, 1], f32, tag="ones")
            nc.gpsimd.memset(ones_t[:], 1.0)

            W_sb = {}
            wall_sb = wpool.tile([128, WCOLS], bf16, tag="Wall")
            nc.sync.dma_start(wall_sb[:], Wallt[:])
            woff = 0
            for cfg in CFGS:
                for w in "ABC":
                    W_sb[f"{w}{cfg.name}"] = wall_sb[:, woff:woff + cfg.GF]
                    woff += cfg.GF

            LUT, LT = {}, {}
            t = wpool.tile([128, 320], bf16, tag="LU0")
            nc.sync.dma_start(t[:], LU0[:])
            LUT["c0"] = t
            t = wpool.tile([128, 3 * 320], bf16, tag="LT1")
            nc.sync.dma_start(t[:], LT1t[:])
            LT["c0"] = LT["c1"] = LUT["c1"] = t
            # big late-use loads go on the sync queue AFTER fc2w so they
            # cannot jump ahead of the head's weights on the DMA engines
            for q in range(2, 4):
                nc.sync.dma_start(
                    fc2w_sb[:, q * 4 * 1280:(q + 1) * 4 * 1280],
                    fc2wt[:, q * 4 * 1280:(q + 1) * 4 * 1280])
            t = wpool.tile([128, 3 * 1280], bf16, tag="LU2")
            nc.sync.dma_start(t[:], LU2t[:])
            LUT["c2"] = t
            t = wpool.tile([128, 10 * 1280], bf16, tag="LT2")
            for q in range(2):
                nc.sync.dma_start(t[:, q * 5 * 1280:(q + 1) * 5 * 1280],
                                  LT2t[:, q * 5 * 1280:(q + 1) * 5 * 1280])
            LT["c2"] = LT["c3"] = LUT["c3"] = t

            # ================= FC head (bf16) =================
            h1T = headp.tile([128, 4 * BL], bf16, tag="h1T")
            ps1 = pslin.tile([128, 512], f32, tag="lin")
            for mt in range(4):
                for kt in range(16):
                    nc.tensor.matmul(
                        ps1[:, mt * BL:(mt + 1) * BL],
                        fc1w_sb[:, kt * 512 + mt * 128: kt * 512 + (mt + 1) * 128],
                        xT_sb[:, kt * BL:(kt + 1) * BL],
                        start=(kt == 0), stop=(kt == 15))
            for mt in range(4):
                nc.scalar.activation(
                    h1T[:, mt * BL:(mt + 1) * BL], ps1[:, mt * BL:(mt + 1) * BL],
                    AF.Relu, bias=fc1b_sb[:, mt:mt + 1])

            # fc2 in 4 column-chunks of 1280 (10 m-tiles each).
            # psum partition = (v0%2)*64+f, col = mi*BL+b ; channels c = v0*64+f.
            # dest: XF0[(b%2)*64+f, (b//2)*80 + v0],  v0 = 2*(mc*10+mi)+p0
            XF0 = poolC.tile([128, 16 * 80], bf16, tag="XF0")
            for mc in range(4):
                ps2 = pslin.tile([128, 512], f32, tag="lin")
                for mi in range(10):
                    for kt in range(4):
                        nc.tensor.matmul(
                            ps2[:, mi * BL:(mi + 1) * BL],
                            fc2w_sb[:, (mc * 4 + kt) * 1280 + mi * 128:
                                    (mc * 4 + kt) * 1280 + (mi + 1) * 128],
                            h1T[:, kt * BL:(kt + 1) * BL],
                            start=(kt == 0), stop=(kt == 3))
                src4 = ps2[:, :10 * BL].rearrange("p (i g j) -> p i g j", g=16, j=2)
                dst4 = XF0[:].rearrange("p (g u q) -> p g u q", u=40, q=2)
                for p0 in range(2):
                    for j in range(2):
                        nc.scalar.activation(
                            dst4[j * 64:(j + 1) * 64, :,
                                 mc * 10:(mc + 1) * 10, p0]
                            .rearrange("p g i -> p i g"),
                            src4[p0 * 64:(p0 + 1) * 64, :, :, j],
                            AF.Copy)

            # ================= cheby layers =================
            XF_cur = XF0
            ar_idx = 0

            for li, cfg in enumerate(CFGS):
                V, Vsp, F = cfg.V, cfg.Vsp, cfg.Fout
                BF = cfg.BF
                last = cfg.name == "c3"

                # --- replicate input for B/A linears if up4 (reuses the
                #     dead fc2w SBUF slot) ---
                if cfg.up4:
                    XFrep = headp.tile([128, cfg.nG * V], bf16, tag="fc2w")
                    s_r = XF_cur[:].rearrange("p (g w) -> p g w", w=Vsp)
                    d_r = XFrep[:].rearrange("p (g w r) -> p g w r",
                                             w=Vsp, r=4)
                    for sp in range(cfg.nVsp):
                        spz = cfg.sps(sp)
                        for r in range(4):
                            dsl = d_r[:, :, sp * 128:sp * 128 + spz, r]
                            ssl = s_r[:, :, sp * 128:sp * 128 + spz]
                            if r % 2 == 0:
                                nc.vector.tensor_copy(dsl, ssl)
                            else:
                                nc.scalar.activation(dsl, ssl, AF.Copy)
                else:
                    XFrep = XF_cur

                def rep_in(g, t, vsz, _X=XFrep, _cfg=cfg):
                    return _X[:, g * _cfg.V + t * 128:
                              g * _cfg.V + t * 128 + vsz]

                # --- C linear (in Vsp space) ---
                XC = poolC.tile([128, cfg.nVsp * BL * F], bf16, tag="XC")
                gpack = max(1, 512 // cfg.GF)
                for s in range(cfg.nVsp):
                    ssz = cfg.sps(s)
                    for g0 in range(0, cfg.nG, gpack):
                        gn = min(gpack, cfg.nG - g0)
                        pc = pslin.tile([128, 512], f32, tag="lin")
                        for gi in range(gn):
                            g = g0 + gi
                            nc.tensor.matmul(
                                pc[:ssz, gi * cfg.GF:(gi + 1) * cfg.GF],
                                XF_cur[:, g * Vsp + s * 128:
                                       g * Vsp + s * 128 + ssz],
                                W_sb[f"C{cfg.name}"],
                                start=True, stop=True)
                        xsl = XC[:ssz, s * BL * F + g0 * cfg.GF:
                                 s * BL * F + (g0 + gn) * cfg.GF]
                        if s % 2 == 0:
                            nc.scalar.activation(
                                xsl, pc[:ssz, :gn * cfg.GF], AF.Copy)
                        else:
                            nc.vector.tensor_copy(xsl, pc[:ssz, :gn * cfg.GF])

                # --- inner = LU @ (2C) + B ;  y = L @ inner + A ---
                Xin = poolB.tile([128, cfg.nVt * BF], bf16, tag="B")
                ytile = poolC.tile([128, cfg.nVt * BF], bf16, tag="YT")

                # per-tile epilogue (interleaved into phase 1 with lag 2 so
                # PE never waits on the psum->sbuf copy): back-transpose to
                # the next level's F-layout + bn_stats, or output staging
                if not last:
                    Gp, nGp = cfg.Gp, cfg.nGp
                    FD = nGp * V
                    nch = cfg.nVt * nGp
                    bnst = miscp.tile([128, nch * 6], f32, tag="bnst")
                    XFn = poolA.tile([128, nGp * V], bf16, tag="A")
                    dstv = XFn[:].rearrange("p (g v) -> p g v", v=V)

                och_group = [None]

                def epilogue(t, _cfg=cfg):
                    vsz = _cfg.vts(t)
                    if last:
                        pt = pstr.tile([128, 1024], bf16, tag="tr")
                        nc.tensor.transpose(
                            pt[:96, :128],
                            ytile[:128, t * BF:(t + 1) * BF],
                            ident_b[:128, :128])
                        if t % 5 == 0:
                            och_group[0] = outp.tile(
                                [BL, 5 * 384], f32, tag="out", name="ochg")
                        och = och_group[0][:, (t % 5) * 384:
                                           (t % 5 + 1) * 384]
                        for fo in range(3):
                            ov = (och.rearrange("b (v f) -> b v f", f=3)
                                  [:, :, fo])
                            pv = pt[fo * 32:(fo + 1) * 32, :128]
                            if fo != 1:
                                nc.vector.tensor_scalar_add(
                                    ov, pv, float(b3_imm[fo]))
                            else:
                                nc.scalar.activation(
                                    ov, pv, AF.Copy,
                                    bias=float(b3_imm[fo]))
                        if t % 5 == 4:
                            nc.sync.dma_start(
                                ydram[:, (t - 4) * 384:(t + 1) * 384],
                                och_group[0][:])
                        return
                    nGp_ = _cfg.nGp
                    for q0 in range(0, nGp_, 8):
                        qn = min(8, nGp_ - q0)
                        pt = pstr.tile([128, 1024], bf16, tag="tr")
                        for qi in range(qn):
                            gp = q0 + qi
                            nc.tensor.transpose(
                                pt[:, qi * 128: qi * 128 + vsz],
                                ytile[:vsz, t * BF + gp * 128:
                                      t * BF + (gp + 1) * 128],
                                ident_b[:vsz, :vsz])
                        dreg = dstv[:, q0:q0 + qn, t * 128:t * 128 + vsz]
                        psrc = (pt[:].rearrange("p (q v) -> p q v", v=128)
                                [:, :qn, :vsz])
                        if (t + q0) % 2 == 0:
                            nc.scalar.activation(dreg, psrc, AF.Copy)
                        else:
                            nc.vector.tensor_copy(dreg, psrc)
                        for qi in range(qn):
                            nc.vector.bn_stats(
                                bnst[:, (t * nGp_ + q0 + qi) * 6:
                                     (t * nGp_ + q0 + qi + 1) * 6],
                                dstv[:, q0 + qi, t * 128:t * 128 + vsz])

                for phase in range(2):
                    srcL = LUT[cfg.name] if phase == 0 else LT[cfg.name]
                    nS = cfg.nVsp if phase == 0 else cfg.nVt
                    ssizes = ([cfg.sps(s) for s in range(nS)] if phase == 0
                              else [cfg.vts(s) for s in range(nS)])
                    rhs = XC if phase == 0 else Xin
                    rhs_w = BL * F if phase == 0 else BF
                    Wacc = W_sb[f"B{cfg.name}" if phase == 0 else f"A{cfg.name}"]
                    dst = Xin if phase == 0 else ytile
                    for t in range(cfg.nVt):
                        vsz = cfg.vts(t)
                        for pc0 in range(0, BF, 1024):
                            pw = min(1024, BF - pc0)
                            pi = psbig.tile([128, max(pw, 512)], f32, tag="big")
                            for nk in range(0, pw, 512):
                                n0 = pc0 + nk
                                n1 = min(n0 + 512, pc0 + pw)
                                for s in range(nS):
                                    ssz = ssizes[s]
                                    nc.tensor.matmul(
                                        pi[:vsz, n0 - pc0:n1 - pc0],
                                        srcL[:ssz, s * V + t * 128:
                                             s * V + t * 128 + vsz],
                                        rhs[:ssz, s * rhs_w + n0:
                                            s * rhs_w + n1],
                                        start=(s == 0), stop=False,
                                        skip_group_check=True)
                                for g in range(n0 // cfg.GF,
                                               (n1 + cfg.GF - 1) // cfg.GF):
                                    nc.tensor.matmul(
                                        pi[:vsz, g * cfg.GF - pc0:
                                           (g + 1) * cfg.GF - pc0],
                                        rep_in(g, t, vsz),
                                        Wacc,
                                        start=False, stop=True,
                                        skip_group_check=True)
                            if last and phase == 1:
                                # reorder (b,fo) -> (fo,b) for output staging
                                nc.vector.tensor_copy(
                                    dst[:vsz, t * BF + pc0: t * BF + pc0 + pw]
                                    .rearrange("p (c b) -> p c b", b=BL),
                                    pi[:vsz, :pw]
                                    .rearrange("p (b c) -> p c b", c=3))
                            else:
                                dsl = dst[:vsz,
                                          t * BF + pc0: t * BF + pc0 + pw]
                                if (t + phase) % 2 == 0:
                                    nc.scalar.activation(
                                        dsl, pi[:vsz, :pw], AF.Copy)
                                else:
                                    nc.vector.tensor_copy(dsl, pi[:vsz, :pw])
                        if phase == 1 and t >= 2:
                            epilogue(t - 2)
                epilogue(cfg.nVt - 2)
                epilogue(cfg.nVt - 1)

                if not last:
                    # --- local partials: stats_l[0, f] = local mu contrib,
                    #     stats_l[0, F+f] = local E[y^2] contrib (the 1/n_g
                    #     factor is folded into sel on host) ---
                    aggr = miscp.tile([128, 2], f32, tag="aggr")
                    nc.vector.bn_aggr(
                        aggr[:], bnst[:].rearrange("p (c s) -> p c s", s=6))
                    part = miscp.tile([128, 1], f32, tag="part")
                    nc.vector.tensor_tensor(
                        out=part[:], in0=aggr[:, 0:1], in1=aggr[:, 0:1],
                        op=ALU.mult)
                    nc.vector.tensor_tensor(
                        out=part[:], in0=part[:], in1=aggr[:, 1:2],
                        op=ALU.add)
                    pst = pslin.tile([128, 512], f32, tag="lin")
                    nc.tensor.matmul(pst[:1, :F], aggr[:, 0:1], sel_sb[F][:],
                                     start=True, stop=True)
                    nc.tensor.matmul(pst[:1, F:2 * F], part[:],
                                     sel_sb[F][:], start=True, stop=True)
                    stats_l = miscp.tile([1, 2 * F], f32, tag="statl")
                    nc.vector.tensor_copy(stats_l[:], pst[:1, :2 * F])
                    bin_ = dramp.tile([1, 2 * F], f32, tag=f"arin{ar_idx}")
                    bout = dramp.tile([NCORES, 2 * F], f32, tag=f"arout{ar_idx}")
                    nc.gpsimd.dma_start(bin_[:], stats_l[:])
                    nc.gpsimd.collective_compute(
                        "AllGather", ALU.bypass,
                        replica_groups=[list(range(NCORES))],
                        ins=[bin_.opt()], outs=[bout.opt()])
                    gsb = miscp.tile([128, 2 * F], f32, tag="gath")
                    nc.sync.dma_start(gsb[:NCORES, :], bout[:])
                    # reduce over cores via PE: mu and E[y^2] per channel,
                    # partition-major [F, 1]
                    psr = pslin.tile([128, 512], f32, tag="lin", name="psr")
                    nc.tensor.matmul(psr[:F, 0:1], gsb[:NCORES, 0:F],
                                     ones_t[:NCORES, :], start=True, stop=True)
                    nc.tensor.matmul(psr[:F, 1:2], gsb[:NCORES, F:2 * F],
                                     ones_t[:NCORES, :], start=True, stop=True)
                    stc = miscp.tile([128, 2], f32, tag=f"stc{ar_idx}")
                    tmp = miscp.tile([128, 2], f32, tag="sttmp")
                    sums = miscp.tile([128, 2], f32, tag="sums")
                    nc.vector.tensor_copy(sums[:F, :], psr[:F, 0:2])
                    # tmp0 = mu^2 ; tmp1 = var = E[y^2] - mu^2
                    nc.vector.tensor_tensor(out=tmp[:F, 0:1],
                                            in0=sums[:F, 0:1],
                                            in1=sums[:F, 0:1], op=ALU.mult)
                    nc.vector.tensor_tensor(out=tmp[:F, 1:2],
                                            in0=sums[:F, 1:2],
                                            in1=tmp[:F, 0:1], op=ALU.subtract)
                    nc.scalar.activation(tmp[:F, 1:2], tmp[:F, 1:2],
                                         AF.Sqrt, bias=eps_t[:F, :])
                    nc.vector.reciprocal(tmp[:F, 1:2], tmp[:F, 1:2])
                    # stc0 = s = rstd*gamma ; stc1 = t = beta - mu*s
                    nc.vector.tensor_tensor(out=stc[:F, 0:1],
                                            in0=tmp[:F, 1:2],
                                            in1=gb_sb[li][:F, 0:1], op=ALU.mult)
                    nc.vector.tensor_tensor(out=tmp[:F, 0:1],
                                            in0=sums[:F, 0:1],
                                            in1=stc[:F, 0:1], op=ALU.mult)
                    nc.vector.tensor_tensor(out=stc[:F, 1:2],
                                            in0=gb_sb[li][:F, 1:2],
                                            in1=tmp[:F, 0:1], op=ALU.subtract)
                    for j in range(1, Gp):
                        nc.vector.tensor_copy(stc[j * F:(j + 1) * F, :],
                                              stc[:F, :])
                    ar_idx += 1
                    # scale+shift+relu per vertex-tile chunk so the next
                    # layer's C-linear (which consumes s-tiles in order)
                    # starts right after chunk 0
                    for s in range(cfg.nVt):
                        vsz = cfg.vts(s)
                        reg = dstv[:, :, s * 128:s * 128 + vsz]
                        if s % 2 == 0:
                            nc.scalar.activation(
                                reg, reg, AF.Relu,
                                scale=stc[:, 0:1], bias=stc[:, 1:2])
                        else:
                            nc.vector.tensor_scalar(
                                reg, reg, stc[:, 0:1], stc[:, 1:2],
                                ALU.mult, ALU.add)
                            nc.vector.tensor_scalar_max(reg, reg, 0.0)
                    XF_cur = XFn

    nc.compile()
    return nc


def kernel(**inputs):
    import sys
    for p in ("/opt/trn_rl_repo", "/opt/trn_rl_repo/concourse"):
        if p not in sys.path:
            sys.path.insert(0, p)
    from concourse.bass_utils import run_bass_kernel_spmd

    host = _build_host(inputs)
    b3 = [float(v) for v in host.pop("b3")]

    key = ("nc",) + tuple(b3)
    if key not in _CACHE:
        _CACHE[key] = _build_nc(b3)
    nc = _CACHE[key]

    import ml_dtypes
    xT = host.pop("xT")
    in_maps = []
    for c in range(NCORES):
        m = dict(host)
        sl = xT[:, c * BL:(c + 1) * BL]  # [2048, BL]
        m["xTt"] = np.ascontiguousarray(
            sl.reshape(16, 128, BL).transpose(1, 0, 2).reshape(
                128, 16 * BL)).astype(ml_dtypes.bfloat16)
        in_maps.append(m)
    res = run_bass_kernel_spmd(nc, in_maps, core_ids=list(range(NCORES)))
    out = np.concatenate(
        [r["y"].reshape(BL, 1280, 3) for r in res.results], axis=0)
    return out.astype(np.float32)


if __name__ == "__main__":
    import reference as R
    inp = R.setup_inputs()
    inp = {k: np.asarray(v) for k, v in inp.items()}
    act = kernel(**inp)
    exp = np.asarray(R.reference(**inp))
    err = np.linalg.norm(act - exp) / np.linalg.norm(exp)
    print("Relative error:", err)


# revision 48
# speedup vs baseline: 1.1158x; 1.0199x over previous
"""Trainium2 Bass kernel for nn_Graph_CNN_Feat_Mesh (Chebyshev GNN decoder).

Strategy (per-core, data-parallel over batch B=256 -> 32/core):
  - All spmms are dense matmuls on the tensor engine (PE) in bf16:
      y = A + L @ (B + L @ (2C)),  A/B/C = feature-space linears of the input.
    L is densified on host; for up4-preceded layers the replication is folded
    into LU = L @ U (contracting the small pre-upsample vertex space), and the
    A/B-linear inputs are read through stride-0 broadcast APs (no copies).
  - B and A linear terms accumulate directly into the spmm PSUM.
  - Activations live in packed F-layout [(j,Fin) partitions, (b//G)*Vsp + v]
    between layers; the per-layer linear emits V-layout directly; one PE
    transpose per layer returns to F-layout, with bn_stats chunks fused in.
  - BatchNorm (training mode, global batch stats) is exact: per-core partial
    (sum, sumsq) go through an AllGather (cheaper than AllReduce) and are
    tree-reduced locally; scale/shift+relu is applied per vertex-tile chunk so
    the next layer's PE work starts immediately.
  - FC head (2048->512->5120) runs fully in bf16 with fp32 PSUM accumulation;
    weights are pre-tiled on host into single-DMA layouts.
"""

import numpy as np

B = 256
NCORES = 8
BL = B // NCORES  # 32
EPS = 1e-5

_CACHE = {}


def _split_W(W):
    W = np.asarray(W, np.float32)
    return W[:, 0::3], W[:, 1::3], W[:, 2::3]


def _dense_L(rows, cols, vals, V):
    L = np.zeros((V, V), np.float32)
    np.add.at(L, (np.asarray(rows), np.asarray(cols)), np.asarray(vals, np.float32))
    return L


def _tile_k(a, tk=128):
    """[K, N] -> [128, (K//128)*N] (k-tile-major columns)."""
    K, N = a.shape
    if K % tk:
        a = np.concatenate([a, np.zeros((tk - K % tk, N), a.dtype)], 0)
    nk = a.shape[0] // tk
    return np.ascontiguousarray(
        a.reshape(nk, tk, N).transpose(1, 0, 2).reshape(tk, nk * N))


class _LCfg:
    def __init__(self, name, Vsp, V, Fin, Fout, up4, bn):
        self.name = name
        self.Vsp = Vsp      # source vertex space of C-linear (pre-up4)
        self.V = V          # output vertex count
        self.Fin = Fin
        self.Fout = Fout
        self.G = 128 // Fin          # batches packed on partitions at input
        self.nG = BL // self.G
        self.GF = self.G * Fout      # N of one B/C/A-linear matmul
        self.Gp = 128 // Fout if Fout in (32, 64) else None
        self.nGp = BL // self.Gp if self.Gp else None
        self.up4 = up4
        self.bn = bn
        self.nVt = (V + 127) // 128
        self.nVsp = (Vsp + 127) // 128
        self.BF = BL * Fout          # free width of V-layout per vtile

    def vts(self, t):
        return min(128, self.V - t * 128)

    def sps(self, s):
        return min(128, self.Vsp - s * 128)


CFGS = [
    _LCfg("c0", 80, 320, 64, 64, True, True),
    _LCfg("c1", 320, 320, 64, 32, False, True),
    _LCfg("c2", 320, 1280, 32, 32, True, True),
    _LCfg("c3", 1280, 1280, 32, 3, False, False),
]


def _wbd(W, G, Fin, Fout, which):
    """Block-diagonal rhs weight [128, G*Fout] for the fused linear.
    which: 'A' -> W0 - W2, 'B' -> W1, 'C' -> 2*W2.  col = j*Fout + c."""
    W0, W1, W2 = _split_W(W)
    M = {"A": W0 - W2, "B": W1, "C": 2.0 * W2}[which]  # [Fout, Fin]
    out = np.zeros((128, G * Fout), np.float32)
    for j in range(G):
        out[j * Fin:(j + 1) * Fin, j * Fout:(j + 1) * Fout] = M.T
    return out


def _build_host(inputs):
    import ml_dtypes
    bf = ml_dtypes.bfloat16
    f32 = np.float32
    d = {}
    xT = np.ascontiguousarray(np.asarray(inputs["x"], f32).T)  # [2048, 256]
    d["xT"] = xT  # sliced + tiled per-core in kernel()
    d["fc1wt"] = _tile_k(np.asarray(inputs["fc1_w"], f32).T).astype(bf)
    d["fc1b"] = np.ascontiguousarray(
        np.asarray(inputs["fc1_b"], f32).reshape(4, 128).T)  # [128,4]
    # fc2: [512, 5120] -> per-mc [512, 1280] k-tiled, mc-major concat
    fc2 = np.asarray(inputs["fc2_w"], f32).T
    d["fc2wt"] = np.concatenate(
        [_tile_k(fc2[:, mc * 1280:(mc + 1) * 1280]) for mc in range(4)],
        axis=1).astype(bf)  # [128, 4*4*1280]

    L1 = _dense_L(inputs["L1_rows"], inputs["L1_cols"], inputs["L1_vals"], 320)
    L2 = _dense_L(inputs["L2_rows"], inputs["L2_cols"], inputs["L2_vals"], 1280)
    U1 = np.repeat(np.eye(80, dtype=f32), 4, axis=0)    # [320, 80]
    U2 = np.repeat(np.eye(320, dtype=f32), 4, axis=0)   # [1280, 320]
    lu0 = (L1 @ U1).T  # [80, 320]
    d["LU0"] = np.concatenate(
        [lu0, np.zeros((48, 320), f32)], 0).astype(bf)   # [128, 320]
    d["LT1t"] = _tile_k(L1.T).astype(bf)                 # [128, 3*320]
    d["LU2t"] = _tile_k((L2 @ U2).T).astype(bf)          # [128, 3*1280]
    d["LT2t"] = _tile_k(L2.T).astype(bf)                 # [128, 10*1280]

    Wn = {"c0": "cl0_w", "c1": "cl1_w", "c2": "cl2_w", "c3": "cl3_w"}
    wall = []
    for cfg in CFGS:
        W = np.asarray(inputs[Wn[cfg.name]], f32)
        for which in "ABC":
            wall.append(_wbd(W, cfg.G, cfg.Fin, cfg.Fout, which))
    d["Wall"] = np.concatenate(wall, axis=1).astype(bf)  # [128, 3*(128+64+128+12)]
    d["b3"] = np.asarray(inputs["cl3_b"], f32).copy()

    for i, (g, b) in enumerate([("bn0_g", "bn0_b"), ("bn1_g", "bn1_b"),
                                ("bn2_g", "bn2_b")]):
        F = len(np.asarray(inputs[g]))
        gbp = np.zeros((128, 2), f32)
        gbp[:F, 0] = np.asarray(inputs[g], f32)
        gbp[:F, 1] = np.asarray(inputs[b], f32)
        d[f"gbp{i}"] = gbp  # partition-major [F rows]: (gamma, beta)

    # selection matrices fold the local-mean -> global-sum/n factor:
    # sum_j mean_{p=(j,f)} * FD / n_g  with FD = nGp*V, n_g = B*V
    for F, nGp, nm in [(64, 16, "sel64"), (32, 8, "sel32")]:
        Gp = 128 // F
        sel = np.zeros((128, F), f32)
        for j in range(Gp):
            sel[j * F:(j + 1) * F] += np.eye(F, dtype=f32)
        d[nm] = sel * (nGp / float(B))
    return d


def _build_nc(b3_imm):
    import sys
    for p in ("/opt/trn_rl_repo", "/opt/trn_rl_repo/concourse"):
        if p not in sys.path:
            sys.path.insert(0, p)
    import concourse.bass as bass  # noqa
    import concourse.mybir as mybir
    import concourse.tile as tile
    from concourse import bacc
    from concourse.masks import make_identity

    f32 = mybir.dt.float32
    bf16 = mybir.dt.bfloat16
    AF = mybir.ActivationFunctionType
    ALU = mybir.AluOpType

    nc = bacc.Bacc(None, target_bir_lowering=False)

    xT = nc.dram_tensor("xTt", [128, 16 * BL], bf16, kind="ExternalInput")
    fc1wt = nc.dram_tensor("fc1wt", [128, 16 * 512], bf16, kind="ExternalInput")
    fc1b = nc.dram_tensor("fc1b", [128, 4], f32, kind="ExternalInput")
    fc2wt = nc.dram_tensor("fc2wt", [128, 16 * 1280], bf16, kind="ExternalInput")
    LU0 = nc.dram_tensor("LU0", [128, 320], bf16, kind="ExternalInput")
    LT1t = nc.dram_tensor("LT1t", [128, 3 * 320], bf16, kind="ExternalInput")
    LU2t = nc.dram_tensor("LU2t", [128, 3 * 1280], bf16, kind="ExternalInput")
    LT2t = nc.dram_tensor("LT2t", [128, 10 * 1280], bf16, kind="ExternalInput")
    WCOLS = sum(cfg.GF for cfg in CFGS) * 3
    Wallt = nc.dram_tensor("Wall", [128, WCOLS], bf16, kind="ExternalInput")
    gbs = [nc.dram_tensor(f"gbp{i}", [128, 2], f32, kind="ExternalInput")
           for i in range(3)]
    sel64 = nc.dram_tensor("sel64", [128, 64], f32, kind="ExternalInput")
    sel32 = nc.dram_tensor("sel32", [128, 32], f32, kind="ExternalInput")
    ydram = nc.dram_tensor("y", [BL, 1280 * 3], f32, kind="ExternalOutput")

    with tile.TileContext(nc) as tc:
        with (
            tc.tile_pool(name="const", bufs=1) as constp,
            tc.tile_pool(name="wpool", bufs=1) as wpool,
            tc.tile_pool(name="headp", bufs=1) as headp,
            tc.tile_pool(name="poolA", bufs=2) as poolA,
            tc.tile_pool(name="poolB", bufs=1) as poolB,
            tc.tile_pool(name="poolC", bufs=1) as poolC,
            tc.tile_pool(name="misc", bufs=1) as miscp,
            tc.tile_pool(name="outp", bufs=2) as outp,
            tc.tile_pool(name="pslin", bufs=2, space="PSUM") as pslin,
            tc.tile_pool(name="psbig", bufs=2, space="PSUM") as psbig,
            tc.tile_pool(name="pstr", bufs=2, space="PSUM") as pstr,
            tc.tile_pool(name="dram", bufs=1, space="DRAM") as dramp,
        ):
            # ---- critical-path loads first: FC head operands, chunked so
            #      the PE can chase the DMA stream ----
            xT_sb = headp.tile([128, 16 * BL], bf16, tag="xT")
            nc.sync.dma_start(xT_sb[:], xT[:])
            fc1b_sb = constp.tile([128, 4], f32, tag="fc1b")
            nc.sync.dma_start(fc1b_sb[:], fc1b[:])
            fc1w_sb = headp.tile([128, 16 * 512], bf16, tag="fc1w")
            for q in range(4):
                nc.sync.dma_start(
                    fc1w_sb[:, q * 4 * 512:(q + 1) * 4 * 512],
                    fc1wt[:, q * 4 * 512:(q + 1) * 4 * 512])
            fc2w_sb = headp.tile([128, 16 * 1280], bf16, tag="fc2w")
            for q in range(2):
                nc.sync.dma_start(
                    fc2w_sb[:, q * 4 * 1280:(q + 1) * 4 * 1280],
                    fc2wt[:, q * 4 * 1280:(q + 1) * 4 * 1280])

            # ---- secondary loads (stream while head computes); everything
            #      rides the in-order sync queue BEHIND the head weights ----
            ident_b = constp.tile([128, 128], bf16, tag="identb")
            make_identity(nc, ident_b[:])
            eps_t = constp.tile([128, 1], f32, tag="eps")
            nc.gpsimd.memset(eps_t[:], EPS)
            ones_t = constp.tile([128, 1], f32, tag="ones")
            nc.gpsimd.memset(ones_t[:], 1.0)
            for q in range(2, 4):
                nc.sync.dma_start(
                    fc2w_sb[:, q * 4 * 1280:(q + 1) * 4 * 1280],
                    fc2wt[:, q * 4 * 1280:(q + 1) * 4 * 1280])

            W_sb = {}
            wall_sb = wpool.tile([128, WCOLS], bf16, tag="Wall")
            nc.sync.dma_start(wall_sb[:], Wallt[:])
            woff = 0
            for cfg in CFGS:
                for w in "ABC":
                    W_sb[f"{w}{cfg.name}"] = wall_sb[:, woff:woff + cfg.GF]
                    woff += cfg.GF

            LUT, LT = {}, {}
            t = wpool.tile([128, 320], bf16, tag="LU0")
            nc.sync.dma_start(t[:], LU0[:])
            LUT["c0"] = t
            t = wpool.tile([128, 3 * 320], bf16, tag="LT1")
            nc.sync.dma_start(t[:], LT1t[:])
            LT["c0"] = LT["c1"] = LUT["c1"] = t
            sel_sb = {64: constp.tile([128, 64], f32, tag="sel64", name="sel64sb"),
                      32: constp.tile([128, 32], f32, tag="sel32", name="sel32sb")}
            nc.sync.dma_start(sel_sb[64][:], sel64[:])
            nc.sync.dma_start(sel_sb[32][:], sel32[:])
            gb_sb = []
            for i in range(3):
                t = constp.tile([128, 2], f32, tag=f"gb{i}")
                nc.sync.dma_start(t[:], gbs[i][:])
                gb_sb.append(t)
            t = wpool.tile([128, 3 * 1280], bf16, tag="LU2")
            nc.sync.dma_start(t[:], LU2t[:])
            LUT["c2"] = t
            t = wpool.tile([128, 10 * 1280], bf16, tag="LT2")
            for q in range(2):
                nc.sync.dma_start(t[:, q * 5 * 1280:(q + 1) * 5 * 1280],
                                  LT2t[:, q * 5 * 1280:(q + 1) * 5 * 1280])
            LT["c2"] = LT["c3"] = LUT["c3"] = t

            # ================= FC head (bf16) =================
            h1T = headp.tile([128, 4 * BL], bf16, tag="h1T")
            ps1 = pslin.tile([128, 512], f32, tag="lin")
            for mt in range(4):
                for kt in range(16):
                    nc.tensor.matmul(
                        ps1[:, mt * BL:(mt + 1) * BL],
                        fc1w_sb[:, kt * 512 + mt * 128: kt * 512 + (mt + 1) * 128],
                        xT_sb[:, kt * BL:(kt + 1) * BL],
                        start=(kt == 0), stop=(kt == 15))
            for mt in range(4):
                nc.scalar.activation(
                    h1T[:, mt * BL:(mt + 1) * BL], ps1[:, mt * BL:(mt + 1) * BL],
                    AF.Relu, bias=fc1b_sb[:, mt:mt + 1])

            # fc2 in 4 column-chunks of 1280 (10 m-tiles each).
            # psum partition = (v0%2)*64+f, col = mi*BL+b ; channels c = v0*64+f.
            # dest: XF0[(b%2)*64+f, (b//2)*80 + v0],  v0 = 2*(mc*10+mi)+p0
            XF0 = poolC.tile([128, 16 * 80], bf16, tag="XF0")
            for mc in range(4):
                ps2 = pslin.tile([128, 512], f32, tag="lin")
                for mi in range(10):
                    for kt in range(4):
                        nc.tensor.matmul(
                            ps2[:, mi * BL:(mi + 1) * BL],
                            fc2w_sb[:, (mc * 4 + kt) * 1280 + mi * 128:
                                    (mc * 4 + kt) * 1280 + (mi + 1) * 128],
                            h1T[:, kt * BL:(kt + 1) * BL],
                            start=(kt == 0), stop=(kt == 3))
                src4 = ps2[:, :10 * BL].rearrange("p (i g j) -> p i g j", g=16, j=2)
                dst4 = XF0[:].rearrange("p (g u q) -> p g u q", u=40, q=2)
                for p0 in range(2):
                    for j in range(2):
                        nc.scalar.activation(
                            dst4[j * 64:(j + 1) * 64, :,
                                 mc * 10:(mc + 1) * 10, p0]
                            .rearrange("p g i -> p i g"),
                            src4[p0 * 64:(p0 + 1) * 64, :, :, j],
                            AF.Copy)

            # ================= cheby layers =================
            XF_cur = XF0
            ar_idx = 0

            for li, cfg in enumerate(CFGS):
                V, Vsp, F = cfg.V, cfg.Vsp, cfg.Fout
                BF = cfg.BF
                last = cfg.name == "c3"

                # --- replicate input for B/A linears if up4 (reuses the
                #     dead fc2w SBUF slot) ---
                if cfg.up4:
                    XFrep = headp.tile([128, cfg.nG * V], bf16, tag="fc2w")
                    s_r = XF_cur[:].rearrange("p (g w) -> p g w", w=Vsp)
                    d_r = XFrep[:].rearrange("p (g w r) -> p g w r",
                                             w=Vsp, r=4)
                    for sp in range(cfg.nVsp):
                        spz = cfg.sps(sp)
                        for r in range(4):
                            dsl = d_r[:, :, sp * 128:sp * 128 + spz, r]
                            ssl = s_r[:, :, sp * 128:sp * 128 + spz]
                            if r % 2 == 0:
                                nc.vector.tensor_copy(dsl, ssl)
                            else:
                                nc.scalar.activation(dsl, ssl, AF.Copy)
                else:
                    XFrep = XF_cur

                def rep_in(g, t, vsz, _X=XFrep, _cfg=cfg):
                    return _X[:, g * _cfg.V + t * 128:
                              g * _cfg.V + t * 128 + vsz]

                # --- C linear (in Vsp space) ---
                XC = poolC.tile([128, cfg.nVsp * BL * F], bf16, tag="XC")
                gpack = max(1, 512 // cfg.GF)
                for s in range(cfg.nVsp):
                    ssz = cfg.sps(s)
                    for g0 in range(0, cfg.nG, gpack):
                        gn = min(gpack, cfg.nG - g0)
                        pc = pslin.tile([128, 512], f32, tag="lin")
                        for gi in range(gn):
                            g = g0 + gi
                            nc.tensor.matmul(
                                pc[:ssz, gi * cfg.GF:(gi + 1) * cfg.GF],
                                XF_cur[:, g * Vsp + s * 128:
                                       g * Vsp + s * 128 + ssz],
                                W_sb[f"C{cfg.name}"],
                                start=True, stop=True)
                        xsl = XC[:ssz, s * BL * F + g0 * cfg.GF:
                                 s * BL * F + (g0 + gn) * cfg.GF]
                        if s % 2 == 0:
                            nc.scalar.activation(
                                xsl, pc[:ssz, :gn * cfg.GF], AF.Copy)
                        else:
                            nc.vector.tensor_copy(xsl, pc[:ssz, :gn * cfg.GF])

                # --- inner = LU @ (2C) + B ;  y = L @ inner + A ---
                Xin = poolB.tile([128, cfg.nVt * BF], bf16, tag="B")
                ytile = poolC.tile([128, cfg.nVt * BF], bf16, tag="YT")

                # per-tile epilogue (interleaved into phase 1 with lag 2 so
                # PE never waits on the psum->sbuf copy): back-transpose to
                # the next level's F-layout + bn_stats, or output staging
                if not last:
                    Gp, nGp = cfg.Gp, cfg.nGp
                    FD = nGp * V
                    nch = cfg.nVt * nGp
                    bnst = miscp.tile([128, nch * 6], f32, tag="bnst")
                    XFn = poolA.tile([128, nGp * V], bf16, tag="A")
                    dstv = XFn[:].rearrange("p (g v) -> p g v", v=V)

                och_group = [None]

                def epilogue(t, _cfg=cfg):
                    vsz = _cfg.vts(t)
                    if last:
                        pt = pstr.tile([128, 1024], bf16, tag="tr")
                        nc.tensor.transpose(
                            pt[:96, :128],
                            ytile[:128, t * BF:(t + 1) * BF],
                            ident_b[:128, :128])
                        if t % 5 == 0:
                            och_group[0] = outp.tile(
                                [BL, 5 * 384], f32, tag="out", name="ochg")
                        och = och_group[0][:, (t % 5) * 384:
                                           (t % 5 + 1) * 384]
                        for fo in range(3):
                            ov = (och.rearrange("b (v f) -> b v f", f=3)
                                  [:, :, fo])
                            pv = pt[fo * 32:(fo + 1) * 32, :128]
                            if fo != 1:
                                nc.vector.tensor_scalar_add(
                                    ov, pv, float(b3_imm[fo]))
                            else:
                                nc.scalar.activation(
                                    ov, pv, AF.Copy,
                                    bias=float(b3_imm[fo]))
                        if t == 4:
                            nc.sync.dma_start(
                                ydram[:, 0:5 * 384], och_group[0][:])
                        elif t == 8:
                            nc.sync.dma_start(
                                ydram[:, 5 * 384:9 * 384],
                                och_group[0][:, 0:4 * 384])
                        elif t == 9:
                            nc.sync.dma_start(
                                ydram[:, 9 * 384:10 * 384],
                                och_group[0][:, 4 * 384:5 * 384])
                        return
                    nGp_ = _cfg.nGp
                    for q0 in range(0, nGp_, 8):
                        qn = min(8, nGp_ - q0)
                        pt = pstr.tile([128, 1024], bf16, tag="tr")
                        for qi in range(qn):
                            gp = q0 + qi
                            nc.tensor.transpose(
                                pt[:, qi * 128: qi * 128 + vsz],
                                ytile[:vsz, t * BF + gp * 128:
                                      t * BF + (gp + 1) * 128],
                                ident_b[:vsz, :vsz])
                        dreg = dstv[:, q0:q0 + qn, t * 128:t * 128 + vsz]
                        psrc = (pt[:].rearrange("p (q v) -> p q v", v=128)
                                [:, :qn, :vsz])
                        if (t + q0) % 2 == 0:
                            nc.scalar.activation(dreg, psrc, AF.Copy)
                        else:
                            nc.vector.tensor_copy(dreg, psrc)
                        for qi in range(qn):
                            nc.vector.bn_stats(
                                bnst[:, (t * nGp_ + q0 + qi) * 6:
                                     (t * nGp_ + q0 + qi + 1) * 6],
                                dstv[:, q0 + qi, t * 128:t * 128 + vsz])

                for phase in range(2):
                    srcL = LUT[cfg.name] if phase == 0 else LT[cfg.name]
                    nS = cfg.nVsp if phase == 0 else cfg.nVt
                    ssizes = ([cfg.sps(s) for s in range(nS)] if phase == 0
                              else [cfg.vts(s) for s in range(nS)])
                    rhs = XC if phase == 0 else Xin
                    rhs_w = BL * F if phase == 0 else BF
                    Wacc = W_sb[f"B{cfg.name}" if phase == 0 else f"A{cfg.name}"]
                    dst = Xin if phase == 0 else ytile
                    for t in range(cfg.nVt):
                        vsz = cfg.vts(t)
                        for pc0 in range(0, BF, 1024):
                            pw = min(1024, BF - pc0)
                            # c3's psum tiles fit the pslin slot; alternating
                            # pools doubles the effective double-buffer depth
                            if last and t % 2 == 1:
                                pi = pslin.tile([128, 512], f32, tag="lin",
                                                name="pic3")
                            else:
                                pi = psbig.tile([128, max(pw, 512)], f32,
                                                tag="big")
                            for nk in range(0, pw, 512):
                                n0 = pc0 + nk
                                n1 = min(n0 + 512, pc0 + pw)
                                for s in range(nS):
                                    ssz = ssizes[s]
                                    nc.tensor.matmul(
                                        pi[:vsz, n0 - pc0:n1 - pc0],
                                        srcL[:ssz, s * V + t * 128:
                                             s * V + t * 128 + vsz],
                                        rhs[:ssz, s * rhs_w + n0:
                                            s * rhs_w + n1],
                                        start=(s == 0), stop=False,
                                        skip_group_check=True)
                                for g in range(n0 // cfg.GF,
                                               (n1 + cfg.GF - 1) // cfg.GF):
                                    nc.tensor.matmul(
                                        pi[:vsz, g * cfg.GF - pc0:
                                           (g + 1) * cfg.GF - pc0],
                                        rep_in(g, t, vsz),
                                        Wacc,
                                        start=False, stop=True,
                                        skip_group_check=True)
                            if last and phase == 1:
                                # reorder (b,fo) -> (fo,b) for output staging
                                nc.vector.tensor_copy(
                                    dst[:vsz, t * BF + pc0: t * BF + pc0 + pw]
                                    .rearrange("p (c b) -> p c b", b=BL),
                                    pi[:vsz, :pw]
                                    .rearrange("p (b c) -> p c b", c=3))
                            else:
                                dsl = dst[:vsz,
                                          t * BF + pc0: t * BF + pc0 + pw]
                                if (t + phase) % 2 == 0:
                                    nc.scalar.activation(
                                        dsl, pi[:vsz, :pw], AF.Copy)
                                else:
                                    nc.vector.tensor_copy(dsl, pi[:vsz, :pw])
                        if phase == 1 and t >= 2:
                            epilogue(t - 2)
                epilogue(cfg.nVt - 2)
                epilogue(cfg.nVt - 1)

                if not last:
                    # --- local partials: stats_l[0, f] = local mu contrib,
                    #     stats_l[0, F+f] = local E[y^2] contrib (the 1/n_g
                    #     factor is folded into sel on host) ---
                    aggr = miscp.tile([128, 2], f32, tag="aggr")
                    nc.vector.bn_aggr(
                        aggr[:], bnst[:].rearrange("p (c s) -> p c s", s=6))
                    part = miscp.tile([128, 1], f32, tag="part")
                    nc.vector.tensor_tensor(
                        out=part[:], in0=aggr[:, 0:1], in1=aggr[:, 0:1],
                        op=ALU.mult)
                    nc.vector.tensor_tensor(
                        out=part[:], in0=part[:], in1=aggr[:, 1:2],
                        op=ALU.add)
                    pst = pslin.tile([128, 512], f32, tag="lin")
                    nc.tensor.matmul(pst[:1, :F], aggr[:, 0:1], sel_sb[F][:],
                                     start=True, stop=True)
                    nc.tensor.matmul(pst[:1, F:2 * F], part[:],
                                     sel_sb[F][:], start=True, stop=True)
                    stats_l = miscp.tile([1, 2 * F], f32, tag="statl")
                    nc.scalar.activation(stats_l[:], pst[:1, :2 * F], AF.Copy)
                    bin_ = dramp.tile([1, 2 * F], f32, tag=f"arin{ar_idx}")
                    bout = dramp.tile([NCORES, 2 * F], f32, tag=f"arout{ar_idx}")
                    nc.scalar.dma_start(bin_[:], stats_l[:])
                    nc.gpsimd.collective_compute(
                        "AllGather", ALU.bypass,
                        replica_groups=[list(range(NCORES))],
                        ins=[bin_.opt()], outs=[bout.opt()])
                    gsb = miscp.tile([128, 2 * F], f32, tag="gath")
                    nc.sync.dma_start(gsb[:NCORES, :], bout[:])
                    # reduce over cores via PE: mu and E[y^2] per channel,
                    # partition-major [F, 1]
                    psr = pslin.tile([128, 512], f32, tag="lin", name="psr")
                    nc.tensor.matmul(psr[:F, 0:1], gsb[:NCORES, 0:F],
                                     ones_t[:NCORES, :], start=True, stop=True)
                    nc.tensor.matmul(psr[:F, 1:2], gsb[:NCORES, F:2 * F],
                                     ones_t[:NCORES, :], start=True, stop=True)
                    stc = miscp.tile([128, 2], f32, tag=f"stc{ar_idx}")
                    tmp = miscp.tile([128, 2], f32, tag="sttmp")
                    sums = miscp.tile([128, 2], f32, tag="sums")
                    nc.vector.tensor_copy(sums[:F, :], psr[:F, 0:2])
                    # tmp0 = mu^2 ; tmp1 = var = E[y^2] - mu^2
                    nc.vector.tensor_tensor(out=tmp[:F, 0:1],
                                            in0=sums[:F, 0:1],
                                            in1=sums[:F, 0:1], op=ALU.mult)
                    nc.vector.tensor_tensor(out=tmp[:F, 1:2],
                                            in0=sums[:F, 1:2],
                                            in1=tmp[:F, 0:1], op=ALU.subtract)
                    nc.scalar.activation(tmp[:F, 1:2], tmp[:F, 1:2],
                                         AF.Sqrt, bias=eps_t[:F, :])
                    nc.vector.reciprocal(tmp[:F, 1:2], tmp[:F, 1:2])
                    # stc0 = s = rstd*gamma ; stc1 = t = beta - mu*s
                    nc.vector.tensor_tensor(out=stc[:F, 0:1],
                                            in0=tmp[:F, 1:2],
                                            in1=gb_sb[li][:F, 0:1], op=ALU.mult)
                    nc.vector.tensor_tensor(out=tmp[:F, 0:1],
                                            in0=sums[:F, 0:1],
                                            in1=stc[:F, 0:1], op=ALU.mult)
                    nc.vector.tensor_tensor(out=stc[:F, 1:2],
                                            in0=gb_sb[li][:F, 1:2],
                                            in1=tmp[:F, 0:1], op=ALU.subtract)
                    for j in range(1, Gp):
                        nc.vector.tensor_copy(stc[j * F:(j + 1) * F, :],
                                              stc[:F, :])
                    ar_idx += 1
                    # scale+shift+relu per vertex-tile chunk so the next
                    # layer's C-linear (which consumes s-tiles in order)
                    # starts right after chunk 0
                    gh = max(1, nGp // 2)
                    for s in range(cfg.nVt):
                        vsz = cfg.vts(s)
                        for g0 in range(0, nGp, gh):
                            reg = dstv[:, g0:g0 + gh,
                                       s * 128:s * 128 + vsz]
                            if (s + g0 // gh) % 2 == 0:
                                nc.scalar.activation(
                                    reg, reg, AF.Relu,
                                    scale=stc[:, 0:1], bias=stc[:, 1:2])
                            else:
                                nc.vector.tensor_scalar(
                                    reg, reg, stc[:, 0:1], stc[:, 1:2],
                                    ALU.mult, ALU.add)
                                nc.vector.tensor_scalar_max(reg, reg, 0.0)
                    XF_cur = XFn

    nc.compile()
    return nc


def kernel(**inputs):
    import sys
    for p in ("/opt/trn_rl_repo", "/opt/trn_rl_repo/concourse"):
        if p not in sys.path:
            sys.path.insert(0, p)
    from concourse.bass_utils import run_bass_kernel_spmd

    host = _build_host(inputs)
    b3 = [float(v) for v in host.pop("b3")]

    key = ("nc",) + tuple(b3)
    if key not in _CACHE:
        _CACHE[key] = _build_nc(b3)
    nc = _CACHE[key]

    import ml_dtypes
    xT = host.pop("xT")
    in_maps = []
    for c in range(NCORES):
        m = dict(host)
        sl = xT[:, c * BL:(c + 1) * BL]  # [2048, BL]
        m["xTt"] = np.ascontiguousarray(
            sl.reshape(16, 128, BL).transpose(1, 0, 2).reshape(
                128, 16 * BL)).astype(ml_dtypes.bfloat16)
        in_maps.append(m)
    res = run_bass_kernel_spmd(nc, in_maps, core_ids=list(range(NCORES)))
    out = np.concatenate(
        [r["y"].reshape(BL, 1280, 3) for r in res.results], axis=0)
    return out.astype(np.float32)


if __name__ == "__main__":
    import reference as R
    inp = R.setup_inputs()
    inp = {k: np.asarray(v) for k, v in inp.items()}
    act = kernel(**inp)
    exp = np.asarray(R.reference(**inp))
    err = np.linalg.norm(act - exp) / np.linalg.norm(exp)
    print("Relative error:", err)
